# revision 14
# baseline (speedup 1.0000x reference)
"""Trainium2 Bass kernel for the ragged bag-of-words MLP model.

Strategy (data-parallel over 8 NeuronCores):
  - Each core owns 512 of the 4096 segments (batch rows). It gathers the
    embedding rows for its ~65536 tokens with `dma_gather` (custom GPSIMD
    ucode; int16 indices, so the 100k vocab is split into 4 contiguous
    buckets of <=32768 rows and tokens are bucket-sorted on the host).
  - Segment-sum is fused into the PE: for every 128-token chunk a one-hot
    selection matrix (seg-id vs iota, DVE is_equal) routes each token row
    to its segment row of a PSUM accumulator; division by counts is a
    per-partition tensor_scalar afterwards.
  - The per-core [512, 256] bag-of-words block is transposed (PE) to
    feature-major and AllGather'd so every core holds the full [256, 4096]
    activation; the 3-layer MLP + BatchNorm(train stats, via bn_stats /
    bn_aggr) + ReLU + logits + BCE loss are then computed redundantly on
    every core (no further collectives needed).
Outputs: loss [1,1] (same on every core) and z packed [128, 32]
(z_full[128*j + p] = z[p, j]); the host unpacks/concatenates.
"""

import sys

sys.path.insert(0, "/opt/trn_rl_repo")

import numpy as np

import concourse.bacc as bacc
import concourse.tile as tile
from concourse import mybir
from concourse.bass_utils import run_bass_kernel_spmd

F32 = mybir.dt.float32
F16 = mybir.dt.float16
I16 = mybir.dt.int16

VOCAB = 100000
H = 256
B = 4096
N_CORES = 8
SEGS_PER_CORE = B // N_CORES          # 512
BLOCKS = SEGS_PER_CORE // 128         # 4 blocks of 128 segments
BUCKET = 32768                        # int16-addressable vocab bucket
N_BUCKETS = (VOCAB + BUCKET - 1) // BUCKET  # 4
MAX_CHUNKS_PER_CALL = 32              # 4096 tokens / call
BN_EPS = 1e-5

_PROGRAM_CACHE: dict = {}


# --------------------------------------------------------------------------
# host-side packing
# --------------------------------------------------------------------------

def _split_tokens(token_ids, segment_ids):
    """Per core, per block, per bucket: (rel_idx int64 array, seg_local f32)."""
    out = []
    for c in range(N_CORES):
        seg0 = c * SEGS_PER_CORE
        lo = np.searchsorted(segment_ids, seg0, "left")
        hi = np.searchsorted(segment_ids, seg0 + SEGS_PER_CORE, "left")
        tok_c = token_ids[lo:hi]
        seg_c = segment_ids[lo:hi] - seg0
        blocks = []
        for b in range(BLOCKS):
            blo = np.searchsorted(seg_c, b * 128, "left")
            bhi = np.searchsorted(seg_c, (b + 1) * 128, "left")
            tok_b = tok_c[blo:bhi]
            seg_b = (seg_c[blo:bhi] - b * 128).astype(np.float32)
            buck = tok_b >> 15
            order = np.argsort(buck, kind="stable")
            tok_b, seg_b, buck = tok_b[order], seg_b[order], buck[order]
            per_bucket = []
            for r in range(N_BUCKETS):
                m = buck == r
                per_bucket.append((tok_b[m] - r * BUCKET, seg_b[m]))
            blocks.append(per_bucket)
        out.append(blocks)
    return out


def _make_schedule(split):
    """Unified call schedule: per block, list of (bucket, n_chunks<=16)."""
    sched = []
    for b in range(BLOCKS):
        calls = []
        for r in range(N_BUCKETS):
            nch = max(
                (len(split[c][b][r][0]) + 127) // 128 for c in range(N_CORES))
            pos = 0
            while pos < nch:
                take = min(MAX_CHUNKS_PER_CALL, nch - pos)
                calls.append((r, take))
                pos += take
        sched.append(calls)
    return sched


def _pack_core(split_c, sched):
    """Build idx16 [128, icols] and sego [128, ccols] for one core."""
    idx_cols, sego_cols = [], []
    for b in range(BLOCKS):
        consumed = {r: 0 for r in range(N_BUCKETS)}
        for (r, take) in sched[b]:
            tok, seg = split_c[b][r]
            s, n = consumed[r] * 128, take * 128
            ti = tok[s:s + n]
            si = seg[s:s + n]
            npad = n - len(ti)
            if npad:
                ti = np.concatenate([ti, np.zeros(npad, ti.dtype)])
                si = np.concatenate([si, np.full(npad, -1.0, np.float32)])
            # dma_gather index wrapping: index i -> [i % 16, i // 16],
            # replicated across the 8 groups of 16 partitions
            w = ti.astype(np.int16).reshape(n // 16, 16).T     # [16, n/16]
            idx_cols.append(np.tile(w, (8, 1)))
            # chunk j, lane p holds token i = j*128 + p
            sego_cols.append(si.reshape(take, 128).T)          # [128, take]
            consumed[r] += take
    idx16 = np.ascontiguousarray(np.concatenate(idx_cols, axis=1))
    sego = np.ascontiguousarray(np.concatenate(sego_cols, axis=1))
    return idx16, sego


# --------------------------------------------------------------------------
# device program
# --------------------------------------------------------------------------

def _build_program(schedule, idxcols, segocols):
    nc = bacc.Bacc("TRN2", target_bir_lowering=False, debug=False,
                   num_devices=N_CORES, num_swdge_queues=4)

    emb = nc.dram_tensor("emb", [VOCAB, H], F16, kind="ExternalInput")
    idx_d = nc.dram_tensor("idx16", [128, idxcols], I16, kind="ExternalInput")
    sego_d = nc.dram_tensor("sego", [128, segocols], F32, kind="ExternalInput")
    invc_d = nc.dram_tensor("invc", [128, BLOCKS], F32, kind="ExternalInput")
    t_d = nc.dram_tensor("tpk", [128, B // 128], F32, kind="ExternalInput")
    w_d = [nc.dram_tensor(f"w{l}", [H, H], F32, kind="ExternalInput")
           for l in range(3)]
    wo_d = nc.dram_tensor("wo", [H, 1], F32, kind="ExternalInput")
    bnp_d = nc.dram_tensor("bnp", [128, 16], F32, kind="ExternalInput")
    iota_d = nc.dram_tensor("iota", [128, 128], F32, kind="ExternalInput")
    ident_d = nc.dram_tensor("ident", [128, 128], F32, kind="ExternalInput")
    z_d = nc.dram_tensor("z", [128, B // 128], F32, kind="ExternalOutput")
    loss_d = nc.dram_tensor("loss", [1, 1], F32, kind="ExternalOutput")

    NB = B // 128  # 32 batch column-chunks of 128

    with tile.TileContext(nc) as tc:
        with (
            tc.tile_pool(name="const", bufs=1) as constp,
            tc.tile_pool(name="gat", bufs=4) as gatp,
            tc.tile_pool(name="sel", bufs=8) as selp,
            tc.tile_pool(name="work", bufs=2) as workp,
            tc.tile_pool(name="act", bufs=2) as actp,
            tc.tile_pool(name="ybuf", bufs=1) as ybufp,
            tc.tile_pool(name="psum", bufs=2, space="PSUM") as psp,
            tc.tile_pool(name="psum1", bufs=1, space="PSUM") as psp1,
            tc.tile_pool(name="dram", bufs=1, space="DRAM") as dramp,
        ):
            # ---- constants / parameters into SBUF
            idx_t = constp.tile([128, idxcols], I16)
            nc.sync.dma_start(out=idx_t[:], in_=idx_d[:])
            sego_t = constp.tile([128, segocols], F32)
            nc.sync.dma_start(out=sego_t[:], in_=sego_d[:])
            invc_t = constp.tile([128, BLOCKS], F32)
            nc.sync.dma_start(out=invc_t[:], in_=invc_d[:])
            t_t = constp.tile([128, NB], F32)
            nc.sync.dma_start(out=t_t[:], in_=t_d[:])
            iota_t = constp.tile([128, 128], F32)
            nc.sync.dma_start(out=iota_t[:], in_=iota_d[:])
            ident_t = constp.tile([128, 128], F32)
            nc.sync.dma_start(out=ident_t[:], in_=ident_d[:])
            bnp_t = constp.tile([128, 16], F32)
            nc.sync.dma_start(out=bnp_t[:], in_=bnp_d[:])
            w_t = []
            for l in range(3):
                wa = constp.tile([128, H], F32, name=f"w{l}a")
                nc.sync.dma_start(out=wa[:], in_=w_d[l][0:128, :])
                wb = constp.tile([128, H], F32, name=f"w{l}b")
                nc.sync.dma_start(out=wb[:], in_=w_d[l][128:256, :])
                w_t.append((wa, wb))
            wo_t = constp.tile([128, 2], F32)
            nc.sync.dma_start(out=wo_t[:, 0:1], in_=wo_d[0:128, :])
            nc.sync.dma_start(out=wo_t[:, 1:2], in_=wo_d[128:256, :])
            ones_t = constp.tile([128, 1], F32)
            nc.vector.memset(ones_t[:], 1.0)

            # ---- stage 1: gather + segment-mean, per block of 128 segments
            xsh = [workp.tile([128, SEGS_PER_CORE], F32, name=f"xsh{h}", bufs=1)
                   for h in range(2)]  # feature-major bow shard
            icol = 0   # running idx16 column offset (n_idxs/16 per call)
            ccol = 0   # running chunk column offset
            call_no = 0
            for b in range(BLOCKS):
                calls = schedule[b]
                nch_total = sum(c[1] for c in calls)
                bow_psA = psp.tile([128, H], F32, space="PSUM", tag="bowA",
                                   bufs=1)
                bow_psB = psp.tile([128, H], F32, space="PSUM", tag="bowB",
                                   bufs=1)
                n_even = (nch_total + 1) // 2
                n_odd = nch_total // 2
                chunk = 0
                for (r, take) in calls:
                    n_i = take * 128
                    gat = gatp.tile([128, take, H], F16, tag="gat")
                    base = r * BUCKET
                    rows = min(BUCKET, VOCAB - base)
                    nc.gpsimd.dma_gather(
                        gat[:],
                        emb[base:base + rows, :],
                        idx_t[:, icol:icol + n_i // 16],
                        n_i,
                        n_i,
                        H,
                        single_packet=False,
                        queue_num=call_no % 4,
                    )
                    call_no += 1
                    icol += n_i // 16
                    for j in range(take):
                        sel = selp.tile([128, 128], F16, tag="sel")
                        nc.vector.tensor_tensor(
                            out=sel[:],
                            in0=sego_t[:, ccol:ccol + 1].to_broadcast([128, 128]),
                            in1=iota_t[:],
                            op=mybir.AluOpType.is_equal,
                        )
                        par = chunk % 2
                        tgt = bow_psA if par == 0 else bow_psB
                        k = chunk // 2
                        nlast = (n_even if par == 0 else n_odd) - 1
                        nc.tensor.matmul(
                            out=tgt[:],
                            lhsT=sel[:],
                            rhs=gat[:, j, :],
                            start=(k == 0),
                            stop=(k == nlast),
                        )
                        ccol += 1
                        chunk += 1
                # mean + transpose to feature-major shard columns
                bow_b = workp.tile([128, H], F32, tag="bow_b")
                nc.vector.tensor_copy(out=bow_b[:], in_=bow_psB[:])
                bow_sum = workp.tile([128, H], F32, tag="bow_sum")
                nc.vector.tensor_tensor(
                    out=bow_sum[:], in0=bow_psA[:], in1=bow_b[:],
                    op=mybir.AluOpType.add,
                )
                bow_sb = workp.tile([128, H], F32, tag="bow_sb")
                nc.vector.tensor_scalar(
                    out=bow_sb[:], in0=bow_sum[:],
                    scalar1=invc_t[:, b:b + 1], scalar2=None,
                    op0=mybir.AluOpType.mult,
                )
                for h in range(2):
                    tp_ps = psp.tile([128, 128], F32, space="PSUM", tag="tp")
                    nc.tensor.transpose(
                        out=tp_ps[:], in_=bow_sb[:, h * 128:(h + 1) * 128],
                        identity=ident_t[:],
                    )
                    nc.vector.tensor_copy(
                        out=xsh[h][:, b * 128:(b + 1) * 128], in_=tp_ps[:])

            # ---- stage 2: AllGather bow across the 8 cores
            ag_in = dramp.tile([2, 128, SEGS_PER_CORE], F32)
            for h in range(2):
                nc.sync.dma_start(out=ag_in[h], in_=xsh[h][:])
            ag_out = dramp.tile([N_CORES, 2, 128, SEGS_PER_CORE], F32,
                                addr_space="Shared")
            nc.gpsimd.collective_compute(
                "AllGather",
                mybir.AluOpType.bypass,
                replica_groups=[list(range(N_CORES))],
                ins=[ag_in.opt()],
                outs=[ag_out.opt()],
            )
            x0 = actp.tile([128, B], F32, name="x0", tag="actH0")
            x1 = actp.tile([128, B], F32, name="x1", tag="actH1")
            for r in range(N_CORES):
                s = slice(r * SEGS_PER_CORE, (r + 1) * SEGS_PER_CORE)
                nc.sync.dma_start(out=x0[:, s], in_=ag_out[r, 0])
                nc.sync.dma_start(out=x1[:, s], in_=ag_out[r, 1])

            # ---- stage 3: MLP with BatchNorm(training stats) + ReLU
            xs = (x0, x1)
            for l in range(3):
                wa, wb = w_t[l]
                ysb = [ybufp.tile([128, B], F32, name=f"y{l}{m}", tag=f"ysb{m}")
                       for m in range(2)]
                stats = workp.tile([128, 2, 8, 6], F32, tag="stats")
                for m in range(2):
                    for ns in range(8):
                        cs = slice(ns * 512, (ns + 1) * 512)
                        y_ps = psp.tile([128, 512], F32, space="PSUM", tag="y")
                        nc.tensor.matmul(
                            out=y_ps[:], lhsT=wa[:, m * 128:(m + 1) * 128],
                            rhs=xs[0][:, cs], start=True, stop=False)
                        nc.tensor.matmul(
                            out=y_ps[:], lhsT=wb[:, m * 128:(m + 1) * 128],
                            rhs=xs[1][:, cs], start=False, stop=True)
                        nc.vector.tensor_copy(out=ysb[m][:, cs], in_=y_ps[:])
                        nc.vector.bn_stats(
                            out=stats[:, m, ns, :], in_=ysb[m][:, cs])
                xn = [actp.tile([128, B], F32, name=f"xn{l}{m}", tag=f"actH{m}")
                      for m in range(2)]
                for m in range(2):
                    mv = workp.tile([128, 2], F32, tag="mv")
                    nc.vector.bn_aggr(out=mv[:], in_=stats[:, m, :, :])
                    std = workp.tile([128, 1], F32, tag="std")
                    nc.scalar.activation(
                        out=std[:], in_=mv[:, 1:2],
                        func=mybir.ActivationFunctionType.Sqrt,
                        bias=bnp_t[:, 13:14])
                    rstd = workp.tile([128, 1], F32, tag="rstd")
                    nc.vector.reciprocal(out=rstd[:], in_=std[:])
                    scale_v = workp.tile([128, 1], F32, tag="scale_v")
                    nc.vector.tensor_tensor(
                        out=scale_v[:], in0=rstd[:],
                        in1=bnp_t[:, 4 * l + m:4 * l + m + 1],
                        op=mybir.AluOpType.mult)
                    mus = workp.tile([128, 1], F32, tag="mus")
                    nc.vector.tensor_tensor(
                        out=mus[:], in0=mv[:, 0:1], in1=scale_v[:],
                        op=mybir.AluOpType.mult)
                    shift_v = workp.tile([128, 1], F32, tag="shift_v")
                    nc.vector.tensor_tensor(
                        out=shift_v[:], in0=bnp_t[:, 4 * l + 2 + m:4 * l + 3 + m],
                        in1=mus[:], op=mybir.AluOpType.subtract)
                    for ns in range(8):
                        cs = slice(ns * 512, (ns + 1) * 512)
                        nc.scalar.activation(
                            out=xn[m][:, cs], in_=ysb[m][:, cs],
                            func=mybir.ActivationFunctionType.Relu,
                            bias=shift_v[:], scale=scale_v[:])
                xs = xn

            # ---- stage 4: logits z[p, j] = z_full[128*j + p]
            z_ps = psp1.tile([128, NB], F32, space="PSUM", tag="z")
            for j in range(NB):
                cs = slice(j * 128, (j + 1) * 128)
                nc.tensor.matmul(out=z_ps[:, j:j + 1], lhsT=xs[0][:, cs],
                                 rhs=wo_t[:, 0:1], start=True, stop=False)
                nc.tensor.matmul(out=z_ps[:, j:j + 1], lhsT=xs[1][:, cs],
                                 rhs=wo_t[:, 1:2], start=False, stop=True)
            z_sb = workp.tile([128, NB], F32, bufs=1)
            nc.scalar.activation(
                out=z_sb[:], in_=z_ps[:],
                func=mybir.ActivationFunctionType.Identity,
                bias=bnp_t[:, 12:13])
            nc.sync.dma_start(out=z_d[:], in_=z_sb[:])

            # ---- stage 5: BCE-with-logits, mean reduction
            relu_t = workp.tile([128, NB], F32, tag="bce1")
            nc.scalar.activation(out=relu_t[:], in_=z_sb[:],
                                 func=mybir.ActivationFunctionType.Relu)
            abs_t = workp.tile([128, NB], F32, tag="bce2")
            nc.scalar.activation(out=abs_t[:], in_=z_sb[:],
                                 func=mybir.ActivationFunctionType.Abs)
            e_t = workp.tile([128, NB], F32, tag="bce3a")
            nc.scalar.activation(out=e_t[:], in_=abs_t[:],
                                 func=mybir.ActivationFunctionType.Exp,
                                 scale=-1.0)
            sp_t = workp.tile([128, NB], F32, tag="bce3")
            nc.scalar.activation(out=sp_t[:], in_=e_t[:],
                                 func=mybir.ActivationFunctionType.Ln,
                                 bias=1.0)
            zt_t = workp.tile([128, NB], F32, tag="bce4")
            nc.vector.tensor_tensor(out=zt_t[:], in0=z_sb[:], in1=t_t[:],
                                    op=mybir.AluOpType.mult)
            s1_t = workp.tile([128, NB], F32, tag="bce5")
            nc.vector.tensor_tensor(out=s1_t[:], in0=relu_t[:], in1=sp_t[:],
                                    op=mybir.AluOpType.add)
            s2_t = workp.tile([128, NB], F32, tag="bce6")
            nc.vector.tensor_tensor(out=s2_t[:], in0=s1_t[:], in1=zt_t[:],
                                    op=mybir.AluOpType.subtract)
            red_t = workp.tile([128, 1], F32, tag="bce7")
            nc.vector.reduce_sum(out=red_t[:], in_=s2_t[:],
                                 axis=mybir.AxisListType.X)
            l_ps = psp1.tile([1, 1], F32, space="PSUM", tag="l")
            nc.tensor.matmul(out=l_ps[:], lhsT=red_t[:], rhs=ones_t[:],
                             start=True, stop=True)
            loss_sb = workp.tile([1, 1], F32, bufs=1)
            nc.scalar.activation(out=loss_sb[:], in_=l_ps[:],
                                 func=mybir.ActivationFunctionType.Identity,
                                 scale=1.0 / B)
            nc.sync.dma_start(out=loss_d[:], in_=loss_sb[:])

    nc.compile()
    return nc


# --------------------------------------------------------------------------
# entry point
# --------------------------------------------------------------------------

def kernel(token_ids, segment_ids, t, emb,
           W1, b1, g1, be1, W2, b2, g2, be2, W3, b3, g3, be3, Wo, bo,
           **_unused):
    token_ids = np.asarray(token_ids).astype(np.int64)
    segment_ids = np.asarray(segment_ids).astype(np.int64)
    t = np.asarray(t, dtype=np.float32)
    emb = np.ascontiguousarray(np.asarray(emb, dtype=np.float16))
    Ws = [np.ascontiguousarray(np.asarray(w, dtype=np.float32))
          for w in (W1, W2, W3)]
    Wo_np = np.ascontiguousarray(np.asarray(Wo, dtype=np.float32).reshape(H, 1))
    gs = [np.asarray(g, dtype=np.float32) for g in (g1, g2, g3)]
    bes = [np.asarray(be, dtype=np.float32) for be in (be1, be2, be3)]
    bo_np = np.asarray(bo, dtype=np.float32).reshape(-1)

    counts = np.bincount(segment_ids, minlength=B).astype(np.float32)
    invc_full = 1.0 / np.maximum(counts, 1.0)

    split = _split_tokens(token_ids, segment_ids)
    sched = _make_schedule(split)
    packed = [_pack_core(split[c], sched) for c in range(N_CORES)]
    idxcols = packed[0][0].shape[1]
    segocols = packed[0][1].shape[1]

    key = (tuple(tuple(s) for s in sched), idxcols, segocols)
    if key not in _PROGRAM_CACHE:
        _PROGRAM_CACHE.clear()
        _PROGRAM_CACHE[key] = _build_program(sched, idxcols, segocols)
    nc = _PROGRAM_CACHE[key]

    # shared input tensors
    iota = np.ascontiguousarray(
        np.broadcast_to(np.arange(128, dtype=np.float32), (128, 128)))
    ident = np.eye(128, dtype=np.float32)
    tpk = np.ascontiguousarray(t.reshape(B // 128, 128).T)
    bnp = np.zeros((128, 16), np.float32)
    for l in range(3):
        for m in range(2):
            bnp[:, 4 * l + m] = gs[l][m * 128:(m + 1) * 128]
            bnp[:, 4 * l + 2 + m] = bes[l][m * 128:(m + 1) * 128]
    bnp[:, 12] = bo_np[0]
    bnp[:, 13] = BN_EPS

    in_maps = []
    for c in range(N_CORES):
        invc = np.ascontiguousarray(
            invc_full[c * SEGS_PER_CORE:(c + 1) * SEGS_PER_CORE]
            .reshape(BLOCKS, 128).T)
        m = {
            "emb": emb, "idx16": packed[c][0], "sego": packed[c][1],
            "invc": invc, "tpk": tpk, "wo": Wo_np, "bnp": bnp,
            "iota": iota, "ident": ident,
        }
        for l in range(3):
            m[f"w{l}"] = Ws[l]
        in_maps.append(m)

    global _LAST_IN_MAPS
    _LAST_IN_MAPS = in_maps
    res = run_bass_kernel_spmd(nc, in_maps, core_ids=list(range(N_CORES)))
    z_pk = res.results[0]["z"]
    loss = res.results[0]["loss"].reshape(())
    z = np.ascontiguousarray(z_pk.T.reshape(B)).astype(np.float32)
    return np.float32(loss), z


# revision 15
# speedup vs baseline: 1.2362x; 1.2362x over previous
"""Trainium2 Bass kernel for the ragged bag-of-words MLP model.

Strategy (data-parallel over 8 NeuronCores):
  - Each core owns 512 of the 4096 segments (batch rows). It gathers the
    embedding rows for its ~65536 tokens with `dma_gather` (custom GPSIMD
    ucode; int16 indices, so the 100k vocab is split into 4 contiguous
    buckets of <=32768 rows and tokens are bucket-sorted on the host).
  - Segment-sum is fused into the PE: for every 128-token chunk a one-hot
    selection matrix (seg-id vs iota, DVE is_equal) routes each token row
    to its segment row of a PSUM accumulator; division by counts is a
    per-partition tensor_scalar afterwards.
  - The per-core [512, 256] bag-of-words block is transposed (PE) to
    feature-major and AllGather'd so every core holds the full [256, 4096]
    activation; the 3-layer MLP + BatchNorm(train stats, via bn_stats /
    bn_aggr) + ReLU + logits + BCE loss are then computed redundantly on
    every core (no further collectives needed).
Outputs: loss [1,1] (same on every core) and z packed [128, 32]
(z_full[128*j + p] = z[p, j]); the host unpacks/concatenates.
"""

import sys

sys.path.insert(0, "/opt/trn_rl_repo")

import numpy as np

import concourse.bacc as bacc
import concourse.tile as tile
from concourse import mybir
from concourse.bass_utils import run_bass_kernel_spmd

F32 = mybir.dt.float32
F16 = mybir.dt.float16
I16 = mybir.dt.int16

VOCAB = 100000
H = 256
B = 4096
N_CORES = 8
SEGS_PER_CORE = B // N_CORES          # 512
BLOCKS = SEGS_PER_CORE // 128         # 4 blocks of 128 segments
BUCKET = 32768                        # int16-addressable vocab bucket
N_BUCKETS = (VOCAB + BUCKET - 1) // BUCKET  # 4
MAX_CHUNKS_PER_CALL = 32              # 4096 tokens / call
BN_EPS = 1e-5

_PROGRAM_CACHE: dict = {}


# --------------------------------------------------------------------------
# host-side packing
# --------------------------------------------------------------------------

def _split_tokens(token_ids, segment_ids):
    """Per core, per block, per bucket: (rel_idx int64 array, seg_local f32)."""
    out = []
    for c in range(N_CORES):
        seg0 = c * SEGS_PER_CORE
        lo = np.searchsorted(segment_ids, seg0, "left")
        hi = np.searchsorted(segment_ids, seg0 + SEGS_PER_CORE, "left")
        tok_c = token_ids[lo:hi]
        seg_c = segment_ids[lo:hi] - seg0
        blocks = []
        for b in range(BLOCKS):
            blo = np.searchsorted(seg_c, b * 128, "left")
            bhi = np.searchsorted(seg_c, (b + 1) * 128, "left")
            tok_b = tok_c[blo:bhi]
            seg_b = (seg_c[blo:bhi] - b * 128).astype(np.float32)
            buck = tok_b >> 15
            order = np.argsort(buck, kind="stable")
            tok_b, seg_b, buck = tok_b[order], seg_b[order], buck[order]
            per_bucket = []
            for r in range(N_BUCKETS):
                m = buck == r
                per_bucket.append((tok_b[m] - r * BUCKET, seg_b[m]))
            blocks.append(per_bucket)
        out.append(blocks)
    return out


def _make_schedule(split):
    """Unified call schedule: per block, list of (bucket, n_chunks<=16)."""
    sched = []
    for b in range(BLOCKS):
        calls = []
        for r in range(N_BUCKETS):
            nch = max(
                (len(split[c][b][r][0]) + 127) // 128 for c in range(N_CORES))
            pos = 0
            while pos < nch:
                take = min(MAX_CHUNKS_PER_CALL, nch - pos)
                calls.append((r, take))
                pos += take
        sched.append(calls)
    return sched


def _pack_core(split_c, sched):
    """Build idx16 [128, icols] and sego [128, ccols] for one core."""
    idx_cols, sego_cols = [], []
    for b in range(BLOCKS):
        consumed = {r: 0 for r in range(N_BUCKETS)}
        for (r, take) in sched[b]:
            tok, seg = split_c[b][r]
            s, n = consumed[r] * 128, take * 128
            ti = tok[s:s + n]
            si = seg[s:s + n]
            npad = n - len(ti)
            if npad:
                ti = np.concatenate([ti, np.zeros(npad, ti.dtype)])
                si = np.concatenate([si, np.full(npad, -1.0, np.float32)])
            # dma_gather index wrapping: index i -> [i % 16, i // 16],
            # replicated across the 8 groups of 16 partitions
            w = ti.astype(np.int16).reshape(n // 16, 16).T     # [16, n/16]
            idx_cols.append(np.tile(w, (8, 1)))
            # chunk j, lane p holds token i = j*128 + p
            sego_cols.append(si.reshape(take, 128).T)          # [128, take]
            consumed[r] += take
    idx16 = np.ascontiguousarray(np.concatenate(idx_cols, axis=1))
    sego = np.ascontiguousarray(np.concatenate(sego_cols, axis=1))
    return idx16, sego


# --------------------------------------------------------------------------
# device program
# --------------------------------------------------------------------------

def _build_program(schedule, idxcols, segocols):
    nc = bacc.Bacc("TRN2", target_bir_lowering=False, debug=False,
                   num_devices=N_CORES, num_swdge_queues=4)

    emb = nc.dram_tensor("emb", [VOCAB, H], F16, kind="ExternalInput")
    idx_d = nc.dram_tensor("idx16", [128, idxcols], I16, kind="ExternalInput")
    sego_d = nc.dram_tensor("sego", [128, segocols], F32, kind="ExternalInput")
    invc_d = nc.dram_tensor("invc", [128, BLOCKS], F32, kind="ExternalInput")
    t_d = nc.dram_tensor("tpk", [128, B // 128], F32, kind="ExternalInput")
    w_d = [nc.dram_tensor(f"w{l}", [H, H], F32, kind="ExternalInput")
           for l in range(3)]
    wo_d = nc.dram_tensor("wo", [H, 1], F32, kind="ExternalInput")
    bnp_d = nc.dram_tensor("bnp", [128, 16], F32, kind="ExternalInput")
    iota_d = nc.dram_tensor("iota", [128, 128], F32, kind="ExternalInput")
    ident_d = nc.dram_tensor("ident", [128, 128], F32, kind="ExternalInput")
    z_d = nc.dram_tensor("z", [128, B // 128], F32, kind="ExternalOutput")
    loss_d = nc.dram_tensor("loss", [1, 1], F32, kind="ExternalOutput")

    NB = B // 128  # 32 batch column-chunks of 128

    with tile.TileContext(nc) as tc:
        with (
            tc.tile_pool(name="const", bufs=1) as constp,
            tc.tile_pool(name="gat", bufs=4) as gatp,
            tc.tile_pool(name="sel", bufs=8) as selp,
            tc.tile_pool(name="work", bufs=2) as workp,
            tc.tile_pool(name="act", bufs=2) as actp,
            tc.tile_pool(name="ybuf", bufs=1) as ybufp,
            tc.tile_pool(name="psum", bufs=2, space="PSUM") as psp,
            tc.tile_pool(name="psum1", bufs=1, space="PSUM") as psp1,
            tc.tile_pool(name="dram", bufs=1, space="DRAM") as dramp,
        ):
            # ---- constants / parameters into SBUF
            idx_t = constp.tile([128, idxcols], I16)
            nc.sync.dma_start(out=idx_t[:], in_=idx_d[:])
            sego_t = constp.tile([128, segocols], F32)
            nc.sync.dma_start(out=sego_t[:], in_=sego_d[:])
            invc_t = constp.tile([128, BLOCKS], F32)
            nc.sync.dma_start(out=invc_t[:], in_=invc_d[:])
            t_t = constp.tile([128, NB], F32)
            nc.sync.dma_start(out=t_t[:], in_=t_d[:])
            iota_t = constp.tile([128, 128], F32)
            nc.sync.dma_start(out=iota_t[:], in_=iota_d[:])
            ident_t = constp.tile([128, 128], F32)
            nc.sync.dma_start(out=ident_t[:], in_=ident_d[:])
            bnp_t = constp.tile([128, 16], F32)
            nc.sync.dma_start(out=bnp_t[:], in_=bnp_d[:])
            w_t = []
            for l in range(3):
                wa = constp.tile([128, H], F32, name=f"w{l}a")
                nc.sync.dma_start(out=wa[:], in_=w_d[l][0:128, :])
                wb = constp.tile([128, H], F32, name=f"w{l}b")
                nc.sync.dma_start(out=wb[:], in_=w_d[l][128:256, :])
                w_t.append((wa, wb))
            wo_t = constp.tile([128, 2], F32)
            nc.sync.dma_start(out=wo_t[:, 0:1], in_=wo_d[0:128, :])
            nc.sync.dma_start(out=wo_t[:, 1:2], in_=wo_d[128:256, :])
            ones_t = constp.tile([128, 1], F32)
            nc.vector.memset(ones_t[:], 1.0)

            # ---- stage 1: gather + segment-mean, per block of 128 segments
            xsh = [workp.tile([128, SEGS_PER_CORE], F32, name=f"xsh{h}", bufs=1)
                   for h in range(2)]  # feature-major bow shard
            icol = 0   # running idx16 column offset (n_idxs/16 per call)
            ccol = 0   # running chunk column offset
            call_no = 0
            for b in range(BLOCKS):
                calls = schedule[b]
                nch_total = sum(c[1] for c in calls)
                bow_ps = psp.tile([128, H], F32, space="PSUM", tag="bow")
                chunk = 0
                for (r, take) in calls:
                    n_i = take * 128
                    gat = gatp.tile([128, take, H], F16, tag="gat")
                    base = r * BUCKET
                    rows = min(BUCKET, VOCAB - base)
                    nc.gpsimd.dma_gather(
                        gat[:],
                        emb[base:base + rows, :],
                        idx_t[:, icol:icol + n_i // 16],
                        n_i,
                        n_i,
                        H,
                        single_packet=False,
                        queue_num=call_no % 4,
                    )
                    call_no += 1
                    icol += n_i // 16
                    for j in range(take):
                        sel = selp.tile([128, 128], F16, tag="sel")
                        nc.vector.tensor_tensor(
                            out=sel[:],
                            in0=sego_t[:, ccol:ccol + 1].to_broadcast([128, 128]),
                            in1=iota_t[:],
                            op=mybir.AluOpType.is_equal,
                        )
                        nc.tensor.matmul(
                            out=bow_ps[:],
                            lhsT=sel[:],
                            rhs=gat[:, j, :],
                            start=(chunk == 0),
                            stop=(chunk == nch_total - 1),
                        )
                        ccol += 1
                        chunk += 1
                # mean + transpose to feature-major shard columns
                bow_sb = workp.tile([128, H], F32, tag="bow_sb")
                nc.vector.tensor_scalar(
                    out=bow_sb[:], in0=bow_ps[:],
                    scalar1=invc_t[:, b:b + 1], scalar2=None,
                    op0=mybir.AluOpType.mult,
                )
                for h in range(2):
                    tp_ps = psp.tile([128, 128], F32, space="PSUM", tag="tp")
                    nc.tensor.transpose(
                        out=tp_ps[:], in_=bow_sb[:, h * 128:(h + 1) * 128],
                        identity=ident_t[:],
                    )
                    nc.vector.tensor_copy(
                        out=xsh[h][:, b * 128:(b + 1) * 128], in_=tp_ps[:])

            # ---- stage 2: AllGather bow across the 8 cores
            ag_in = dramp.tile([2, 128, SEGS_PER_CORE], F32)
            for h in range(2):
                nc.sync.dma_start(out=ag_in[h], in_=xsh[h][:])
            ag_out = dramp.tile([N_CORES, 2, 128, SEGS_PER_CORE], F32,
                                addr_space="Shared")
            nc.gpsimd.collective_compute(
                "AllGather",
                mybir.AluOpType.bypass,
                replica_groups=[list(range(N_CORES))],
                ins=[ag_in.opt()],
                outs=[ag_out.opt()],
            )
            x0 = actp.tile([128, B], F32, name="x0", tag="actH0")
            x1 = actp.tile([128, B], F32, name="x1", tag="actH1")
            for r in range(N_CORES):
                s = slice(r * SEGS_PER_CORE, (r + 1) * SEGS_PER_CORE)
                nc.sync.dma_start(out=x0[:, s], in_=ag_out[r, 0])
                nc.sync.dma_start(out=x1[:, s], in_=ag_out[r, 1])

            # ---- stage 3: MLP with BatchNorm(training stats) + ReLU
            xs = (x0, x1)
            for l in range(3):
                wa, wb = w_t[l]
                ysb = [ybufp.tile([128, B], F32, name=f"y{l}{m}", tag=f"ysb{m}")
                       for m in range(2)]
                stats = workp.tile([128, 2, 8, 6], F32, tag="stats")
                for m in range(2):
                    for ns in range(8):
                        cs = slice(ns * 512, (ns + 1) * 512)
                        y_ps = psp.tile([128, 512], F32, space="PSUM", tag="y")
                        nc.tensor.matmul(
                            out=y_ps[:], lhsT=wa[:, m * 128:(m + 1) * 128],
                            rhs=xs[0][:, cs], start=True, stop=False)
                        nc.tensor.matmul(
                            out=y_ps[:], lhsT=wb[:, m * 128:(m + 1) * 128],
                            rhs=xs[1][:, cs], start=False, stop=True)
                        nc.vector.tensor_copy(out=ysb[m][:, cs], in_=y_ps[:])
                        nc.vector.bn_stats(
                            out=stats[:, m, ns, :], in_=ysb[m][:, cs])
                xn = [actp.tile([128, B], F32, name=f"xn{l}{m}", tag=f"actH{m}")
                      for m in range(2)]
                for m in range(2):
                    mv = workp.tile([128, 2], F32, tag="mv")
                    nc.vector.bn_aggr(out=mv[:], in_=stats[:, m, :, :])
                    std = workp.tile([128, 1], F32, tag="std")
                    nc.scalar.activation(
                        out=std[:], in_=mv[:, 1:2],
                        func=mybir.ActivationFunctionType.Sqrt,
                        bias=bnp_t[:, 13:14])
                    rstd = workp.tile([128, 1], F32, tag="rstd")
                    nc.vector.reciprocal(out=rstd[:], in_=std[:])
                    scale_v = workp.tile([128, 1], F32, tag="scale_v")
                    nc.vector.tensor_tensor(
                        out=scale_v[:], in0=rstd[:],
                        in1=bnp_t[:, 4 * l + m:4 * l + m + 1],
                        op=mybir.AluOpType.mult)
                    mus = workp.tile([128, 1], F32, tag="mus")
                    nc.vector.tensor_tensor(
                        out=mus[:], in0=mv[:, 0:1], in1=scale_v[:],
                        op=mybir.AluOpType.mult)
                    shift_v = workp.tile([128, 1], F32, tag="shift_v")
                    nc.vector.tensor_tensor(
                        out=shift_v[:], in0=bnp_t[:, 4 * l + 2 + m:4 * l + 3 + m],
                        in1=mus[:], op=mybir.AluOpType.subtract)
                    for ns in range(8):
                        cs = slice(ns * 512, (ns + 1) * 512)
                        nc.scalar.activation(
                            out=xn[m][:, cs], in_=ysb[m][:, cs],
                            func=mybir.ActivationFunctionType.Relu,
                            bias=shift_v[:], scale=scale_v[:])
                xs = xn

            # ---- stage 4: logits z[p, j] = z_full[128*j + p]
            z_ps = psp1.tile([128, NB], F32, space="PSUM", tag="z")
            for j in range(NB):
                cs = slice(j * 128, (j + 1) * 128)
                nc.tensor.matmul(out=z_ps[:, j:j + 1], lhsT=xs[0][:, cs],
                                 rhs=wo_t[:, 0:1], start=True, stop=False)
                nc.tensor.matmul(out=z_ps[:, j:j + 1], lhsT=xs[1][:, cs],
                                 rhs=wo_t[:, 1:2], start=False, stop=True)
            z_sb = workp.tile([128, NB], F32, bufs=1)
            nc.scalar.activation(
                out=z_sb[:], in_=z_ps[:],
                func=mybir.ActivationFunctionType.Identity,
                bias=bnp_t[:, 12:13])
            nc.sync.dma_start(out=z_d[:], in_=z_sb[:])

            # ---- stage 5: BCE-with-logits, mean reduction
            relu_t = workp.tile([128, NB], F32, tag="bce1")
            nc.scalar.activation(out=relu_t[:], in_=z_sb[:],
                                 func=mybir.ActivationFunctionType.Relu)
            abs_t = workp.tile([128, NB], F32, tag="bce2")
            nc.scalar.activation(out=abs_t[:], in_=z_sb[:],
                                 func=mybir.ActivationFunctionType.Abs)
            e_t = workp.tile([128, NB], F32, tag="bce3a")
            nc.scalar.activation(out=e_t[:], in_=abs_t[:],
                                 func=mybir.ActivationFunctionType.Exp,
                                 scale=-1.0)
            sp_t = workp.tile([128, NB], F32, tag="bce3")
            nc.scalar.activation(out=sp_t[:], in_=e_t[:],
                                 func=mybir.ActivationFunctionType.Ln,
                                 bias=1.0)
            zt_t = workp.tile([128, NB], F32, tag="bce4")
            nc.vector.tensor_tensor(out=zt_t[:], in0=z_sb[:], in1=t_t[:],
                                    op=mybir.AluOpType.mult)
            s1_t = workp.tile([128, NB], F32, tag="bce5")
            nc.vector.tensor_tensor(out=s1_t[:], in0=relu_t[:], in1=sp_t[:],
                                    op=mybir.AluOpType.add)
            s2_t = workp.tile([128, NB], F32, tag="bce6")
            nc.vector.tensor_tensor(out=s2_t[:], in0=s1_t[:], in1=zt_t[:],
                                    op=mybir.AluOpType.subtract)
            red_t = workp.tile([128, 1], F32, tag="bce7")
            nc.vector.reduce_sum(out=red_t[:], in_=s2_t[:],
                                 axis=mybir.AxisListType.X)
            l_ps = psp1.tile([1, 1], F32, space="PSUM", tag="l")
            nc.tensor.matmul(out=l_ps[:], lhsT=red_t[:], rhs=ones_t[:],
                             start=True, stop=True)
            loss_sb = workp.tile([1, 1], F32, bufs=1)
            nc.scalar.activation(out=loss_sb[:], in_=l_ps[:],
                                 func=mybir.ActivationFunctionType.Identity,
                                 scale=1.0 / B)
            nc.sync.dma_start(out=loss_d[:], in_=loss_sb[:])

    nc.compile()
    return nc


# --------------------------------------------------------------------------
# entry point
# --------------------------------------------------------------------------

def kernel(token_ids, segment_ids, t, emb,
           W1, b1, g1, be1, W2, b2, g2, be2, W3, b3, g3, be3, Wo, bo,
           **_unused):
    token_ids = np.asarray(token_ids).astype(np.int64)
    segment_ids = np.asarray(segment_ids).astype(np.int64)
    t = np.asarray(t, dtype=np.float32)
    emb = np.ascontiguousarray(np.asarray(emb, dtype=np.float16))
    Ws = [np.ascontiguousarray(np.asarray(w, dtype=np.float32))
          for w in (W1, W2, W3)]
    Wo_np = np.ascontiguousarray(np.asarray(Wo, dtype=np.float32).reshape(H, 1))
    gs = [np.asarray(g, dtype=np.float32) for g in (g1, g2, g3)]
    bes = [np.asarray(be, dtype=np.float32) for be in (be1, be2, be3)]
    bo_np = np.asarray(bo, dtype=np.float32).reshape(-1)

    counts = np.bincount(segment_ids, minlength=B).astype(np.float32)
    invc_full = 1.0 / np.maximum(counts, 1.0)

    split = _split_tokens(token_ids, segment_ids)
    sched = _make_schedule(split)
    packed = [_pack_core(split[c], sched) for c in range(N_CORES)]
    idxcols = packed[0][0].shape[1]
    segocols = packed[0][1].shape[1]

    key = (tuple(tuple(s) for s in sched), idxcols, segocols)
    if key not in _PROGRAM_CACHE:
        _PROGRAM_CACHE.clear()
        _PROGRAM_CACHE[key] = _build_program(sched, idxcols, segocols)
    nc = _PROGRAM_CACHE[key]

    # shared input tensors
    iota = np.ascontiguousarray(
        np.broadcast_to(np.arange(128, dtype=np.float32), (128, 128)))
    ident = np.eye(128, dtype=np.float32)
    tpk = np.ascontiguousarray(t.reshape(B // 128, 128).T)
    bnp = np.zeros((128, 16), np.float32)
    for l in range(3):
        for m in range(2):
            bnp[:, 4 * l + m] = gs[l][m * 128:(m + 1) * 128]
            bnp[:, 4 * l + 2 + m] = bes[l][m * 128:(m + 1) * 128]
    bnp[:, 12] = bo_np[0]
    bnp[:, 13] = BN_EPS

    in_maps = []
    for c in range(N_CORES):
        invc = np.ascontiguousarray(
            invc_full[c * SEGS_PER_CORE:(c + 1) * SEGS_PER_CORE]
            .reshape(BLOCKS, 128).T)
        m = {
            "emb": emb, "idx16": packed[c][0], "sego": packed[c][1],
            "invc": invc, "tpk": tpk, "wo": Wo_np, "bnp": bnp,
            "iota": iota, "ident": ident,
        }
        for l in range(3):
            m[f"w{l}"] = Ws[l]
        in_maps.append(m)

    global _LAST_IN_MAPS
    _LAST_IN_MAPS = in_maps
    res = run_bass_kernel_spmd(nc, in_maps, core_ids=list(range(N_CORES)))
    z_pk = res.results[0]["z"]
    loss = res.results[0]["loss"].reshape(())
    z = np.ascontiguousarray(z_pk.T.reshape(B)).astype(np.float32)
    return np.float32(loss), z


# revision 16
# speedup vs baseline: 1.3936x; 1.1273x over previous
"""Trainium2 Bass kernel for the ragged bag-of-words MLP model.

Strategy (data-parallel over 8 NeuronCores):
  - Each core owns 512 of the 4096 segments (batch rows). It gathers the
    embedding rows for its ~65536 tokens with `dma_gather` (custom GPSIMD
    ucode; int16 indices, so the 100k vocab is split into 4 contiguous
    buckets of <=32768 rows and tokens are bucket-sorted on the host).
  - Segment-sum is fused into the PE: for every 128-token chunk a one-hot
    selection matrix (seg-id vs iota, DVE is_equal) routes each token row
    to its segment row of a PSUM accumulator; division by counts is a
    per-partition tensor_scalar afterwards.
  - The per-core [512, 256] bag-of-words block is transposed (PE) to
    feature-major and AllGather'd so every core holds the full [256, 4096]
    activation; the 3-layer MLP + BatchNorm(train stats, via bn_stats /
    bn_aggr) + ReLU + logits + BCE loss are then computed redundantly on
    every core (no further collectives needed).
Outputs: loss [1,1] (same on every core) and z packed [128, 32]
(z_full[128*j + p] = z[p, j]); the host unpacks/concatenates.
"""

import sys

sys.path.insert(0, "/opt/trn_rl_repo")

import numpy as np

import concourse.bacc as bacc
import concourse.tile as tile
from concourse import mybir
from concourse.bass_utils import run_bass_kernel_spmd

F32 = mybir.dt.float32
F16 = mybir.dt.float16
I16 = mybir.dt.int16

VOCAB = 100000
H = 256
B = 4096
N_CORES = 8
SEGS_PER_CORE = B // N_CORES          # 512
BLOCKS = SEGS_PER_CORE // 128         # 4 blocks of 128 segments
BUCKET = 32768                        # int16-addressable vocab bucket
N_BUCKETS = (VOCAB + BUCKET - 1) // BUCKET  # 4
MAX_CHUNKS_PER_CALL = 32              # 4096 tokens / call
BN_EPS = 1e-5

_PROGRAM_CACHE: dict = {}


# --------------------------------------------------------------------------
# host-side packing
# --------------------------------------------------------------------------

def _split_tokens(token_ids, segment_ids):
    """Per core, per block, per bucket: (rel_idx int64 array, seg_local f32)."""
    out = []
    for c in range(N_CORES):
        seg0 = c * SEGS_PER_CORE
        lo = np.searchsorted(segment_ids, seg0, "left")
        hi = np.searchsorted(segment_ids, seg0 + SEGS_PER_CORE, "left")
        tok_c = token_ids[lo:hi]
        seg_c = segment_ids[lo:hi] - seg0
        blocks = []
        for b in range(BLOCKS):
            blo = np.searchsorted(seg_c, b * 128, "left")
            bhi = np.searchsorted(seg_c, (b + 1) * 128, "left")
            tok_b = tok_c[blo:bhi]
            seg_b = (seg_c[blo:bhi] - b * 128).astype(np.float32)
            buck = tok_b >> 15
            order = np.argsort(buck, kind="stable")
            tok_b, seg_b, buck = tok_b[order], seg_b[order], buck[order]
            per_bucket = []
            for r in range(N_BUCKETS):
                m = buck == r
                per_bucket.append((tok_b[m] - r * BUCKET, seg_b[m]))
            blocks.append(per_bucket)
        out.append(blocks)
    return out


def _make_schedule(split):
    """Unified call schedule: per block, list of (bucket, n_chunks<=16)."""
    sched = []
    for b in range(BLOCKS):
        calls = []
        for r in range(N_BUCKETS):
            nch = max(
                (len(split[c][b][r][0]) + 127) // 128 for c in range(N_CORES))
            pos = 0
            while pos < nch:
                take = min(MAX_CHUNKS_PER_CALL, nch - pos)
                calls.append((r, take))
                pos += take
        sched.append(calls)
    return sched


def _pack_core(split_c, sched):
    """Build idx16 [128, icols] and sego [128, ccols] for one core."""
    idx_cols, sego_cols = [], []
    for b in range(BLOCKS):
        consumed = {r: 0 for r in range(N_BUCKETS)}
        for (r, take) in sched[b]:
            tok, seg = split_c[b][r]
            s, n = consumed[r] * 128, take * 128
            ti = tok[s:s + n]
            si = seg[s:s + n]
            npad = n - len(ti)
            if npad:
                ti = np.concatenate([ti, np.zeros(npad, ti.dtype)])
                si = np.concatenate([si, np.full(npad, -1.0, np.float32)])
            # dma_gather index wrapping: index i -> [i % 16, i // 16],
            # replicated across the 8 groups of 16 partitions
            w = ti.astype(np.int16).reshape(n // 16, 16).T     # [16, n/16]
            idx_cols.append(np.tile(w, (8, 1)))
            # chunk j, lane p holds token i = j*128 + p
            sego_cols.append(si.reshape(take, 128).T)          # [128, take]
            consumed[r] += take
    idx16 = np.ascontiguousarray(np.concatenate(idx_cols, axis=1))
    sego = np.ascontiguousarray(np.concatenate(sego_cols, axis=1))
    return idx16, sego


# --------------------------------------------------------------------------
# device program
# --------------------------------------------------------------------------

def _build_program(schedule, idxcols, segocols):
    nc = bacc.Bacc("TRN2", target_bir_lowering=False, debug=False,
                   num_devices=N_CORES, num_swdge_queues=4)

    emb = nc.dram_tensor("emb", [VOCAB, H], F16, kind="ExternalInput")
    idx_d = nc.dram_tensor("idx16", [128, idxcols], I16, kind="ExternalInput")
    sego_d = nc.dram_tensor("sego", [128, segocols], F32, kind="ExternalInput")
    invc_d = nc.dram_tensor("invc", [128, BLOCKS], F32, kind="ExternalInput")
    t_d = nc.dram_tensor("tpk", [128, B // 128], F32, kind="ExternalInput")
    w_d = [nc.dram_tensor(f"w{l}", [H, H], F32, kind="ExternalInput")
           for l in range(3)]
    wo_d = nc.dram_tensor("wo", [H, 1], F32, kind="ExternalInput")
    bnp_d = nc.dram_tensor("bnp", [128, 16], F32, kind="ExternalInput")
    iota_d = nc.dram_tensor("iota", [128, 1024], F32, kind="ExternalInput")
    ident_d = nc.dram_tensor("ident", [128, 128], F32, kind="ExternalInput")
    z_d = nc.dram_tensor("z", [128, B // 128], F32, kind="ExternalOutput")
    loss_d = nc.dram_tensor("loss", [1, 1], F32, kind="ExternalOutput")

    NB = B // 128  # 32 batch column-chunks of 128

    with tile.TileContext(nc) as tc:
        with (
            tc.tile_pool(name="const", bufs=1) as constp,
            tc.tile_pool(name="gat", bufs=4) as gatp,
            tc.tile_pool(name="sel", bufs=4) as selp,
            tc.tile_pool(name="work", bufs=2) as workp,
            tc.tile_pool(name="act", bufs=2) as actp,
            tc.tile_pool(name="ybuf", bufs=1) as ybufp,
            tc.tile_pool(name="psum", bufs=2, space="PSUM") as psp,
            tc.tile_pool(name="psum1", bufs=1, space="PSUM") as psp1,
            tc.tile_pool(name="dram", bufs=1, space="DRAM") as dramp,
        ):
            # ---- constants / parameters into SBUF
            idx_t = constp.tile([128, idxcols], I16)
            nc.sync.dma_start(out=idx_t[:], in_=idx_d[:])
            sego_t = constp.tile([128, segocols], F32)
            nc.sync.dma_start(out=sego_t[:], in_=sego_d[:])
            invc_t = constp.tile([128, BLOCKS], F32)
            nc.sync.dma_start(out=invc_t[:], in_=invc_d[:])
            t_t = constp.tile([128, NB], F32)
            nc.sync.dma_start(out=t_t[:], in_=t_d[:])
            iota_t = constp.tile([128, 1024], F32)
            nc.sync.dma_start(out=iota_t[:], in_=iota_d[:])
            ident_t = constp.tile([128, 128], F32)
            nc.sync.dma_start(out=ident_t[:], in_=ident_d[:])
            bnp_t = constp.tile([128, 16], F32)
            nc.sync.dma_start(out=bnp_t[:], in_=bnp_d[:])
            w_t = []
            for l in range(3):
                wa = constp.tile([128, H], F32, name=f"w{l}a")
                nc.sync.dma_start(out=wa[:], in_=w_d[l][0:128, :])
                wb = constp.tile([128, H], F32, name=f"w{l}b")
                nc.sync.dma_start(out=wb[:], in_=w_d[l][128:256, :])
                w_t.append((wa, wb))
            wo_t = constp.tile([128, 2], F32)
            nc.sync.dma_start(out=wo_t[:, 0:1], in_=wo_d[0:128, :])
            nc.sync.dma_start(out=wo_t[:, 1:2], in_=wo_d[128:256, :])
            ones_t = constp.tile([128, 1], F32)
            nc.vector.memset(ones_t[:], 1.0)

            # ---- stage 1: gather + segment-mean, per block of 128 segments
            xsh = [workp.tile([128, SEGS_PER_CORE], F32, name=f"xsh{h}", bufs=1)
                   for h in range(2)]  # feature-major bow shard
            icol = 0   # running idx16 column offset (n_idxs/16 per call)
            ccol = 0   # running chunk column offset
            call_no = 0
            for b in range(BLOCKS):
                calls = schedule[b]
                nch_total = sum(c[1] for c in calls)
                bow_ps = psp.tile([128, H], F32, space="PSUM", tag="bow")
                chunk = 0
                for (r, take) in calls:
                    n_i = take * 128
                    gat = gatp.tile([128, take, H], F16, tag="gat")
                    base = r * BUCKET
                    rows = min(BUCKET, VOCAB - base)
                    nc.gpsimd.dma_gather(
                        gat[:],
                        emb[base:base + rows, :],
                        idx_t[:, icol:icol + n_i // 16],
                        n_i,
                        n_i,
                        H,
                        single_packet=False,
                        queue_num=call_no % 4,
                    )
                    call_no += 1
                    icol += n_i // 16
                    for g0 in range(0, take, 8):
                        gn = min(8, take - g0)
                        sel8 = selp.tile([128, 8, 128], F16, tag="sel")
                        nc.vector.tensor_tensor(
                            out=sel8[:, 0:gn, :],
                            in0=sego_t[:, ccol + g0:ccol + g0 + gn]
                            .rearrange("p (c o) -> p c o", o=1)
                            .to_broadcast([128, gn, 128]),
                            in1=iota_t[:, 0:gn * 128]
                            .rearrange("p (c i) -> p c i", c=gn),
                            op=mybir.AluOpType.is_equal,
                        )
                        for jj in range(gn):
                            nc.tensor.matmul(
                                out=bow_ps[:],
                                lhsT=sel8[:, jj, :],
                                rhs=gat[:, g0 + jj, :],
                                start=(chunk == 0),
                                stop=(chunk == nch_total - 1),
                            )
                            chunk += 1
                    ccol += take
                # mean + transpose to feature-major shard columns
                bow_sb = workp.tile([128, H], F32, tag="bow_sb")
                nc.vector.tensor_scalar(
                    out=bow_sb[:], in0=bow_ps[:],
                    scalar1=invc_t[:, b:b + 1], scalar2=None,
                    op0=mybir.AluOpType.mult,
                )
                for h in range(2):
                    tp_ps = psp.tile([128, 128], F32, space="PSUM", tag="tp")
                    nc.tensor.transpose(
                        out=tp_ps[:], in_=bow_sb[:, h * 128:(h + 1) * 128],
                        identity=ident_t[:],
                    )
                    nc.vector.tensor_copy(
                        out=xsh[h][:, b * 128:(b + 1) * 128], in_=tp_ps[:])

            # ---- stage 2: AllGather bow across the 8 cores
            ag_in = dramp.tile([2, 128, SEGS_PER_CORE], F32)
            for h in range(2):
                nc.sync.dma_start(out=ag_in[h], in_=xsh[h][:])
            ag_out = dramp.tile([N_CORES, 2, 128, SEGS_PER_CORE], F32,
                                addr_space="Shared")
            nc.gpsimd.collective_compute(
                "AllGather",
                mybir.AluOpType.bypass,
                replica_groups=[list(range(N_CORES))],
                ins=[ag_in.opt()],
                outs=[ag_out.opt()],
            )
            x0 = actp.tile([128, B], F32, name="x0", tag="actH0")
            x1 = actp.tile([128, B], F32, name="x1", tag="actH1")
            for r in range(N_CORES):
                s = slice(r * SEGS_PER_CORE, (r + 1) * SEGS_PER_CORE)
                nc.sync.dma_start(out=x0[:, s], in_=ag_out[r, 0])
                nc.sync.dma_start(out=x1[:, s], in_=ag_out[r, 1])

            # ---- stage 3: MLP with BatchNorm(training stats) + ReLU
            xs = (x0, x1)
            for l in range(3):
                wa, wb = w_t[l]
                ysb = [ybufp.tile([128, B], F32, name=f"y{l}{m}", tag=f"ysb{m}")
                       for m in range(2)]
                stats = workp.tile([128, 2, 8, 6], F32, tag="stats")
                for m in range(2):
                    for ns in range(8):
                        cs = slice(ns * 512, (ns + 1) * 512)
                        y_ps = psp.tile([128, 512], F32, space="PSUM", tag="y")
                        nc.tensor.matmul(
                            out=y_ps[:], lhsT=wa[:, m * 128:(m + 1) * 128],
                            rhs=xs[0][:, cs], start=True, stop=False)
                        nc.tensor.matmul(
                            out=y_ps[:], lhsT=wb[:, m * 128:(m + 1) * 128],
                            rhs=xs[1][:, cs], start=False, stop=True)
                        nc.vector.tensor_copy(out=ysb[m][:, cs], in_=y_ps[:])
                        nc.vector.bn_stats(
                            out=stats[:, m, ns, :], in_=ysb[m][:, cs])
                xn = [actp.tile([128, B], F32, name=f"xn{l}{m}", tag=f"actH{m}")
                      for m in range(2)]
                for m in range(2):
                    mv = workp.tile([128, 2], F32, tag="mv")
                    nc.vector.bn_aggr(out=mv[:], in_=stats[:, m, :, :])
                    std = workp.tile([128, 1], F32, tag="std")
                    nc.scalar.activation(
                        out=std[:], in_=mv[:, 1:2],
                        func=mybir.ActivationFunctionType.Sqrt,
                        bias=bnp_t[:, 13:14])
                    rstd = workp.tile([128, 1], F32, tag="rstd")
                    nc.vector.reciprocal(out=rstd[:], in_=std[:])
                    scale_v = workp.tile([128, 1], F32, tag="scale_v")
                    nc.vector.tensor_tensor(
                        out=scale_v[:], in0=rstd[:],
                        in1=bnp_t[:, 4 * l + m:4 * l + m + 1],
                        op=mybir.AluOpType.mult)
                    mus = workp.tile([128, 1], F32, tag="mus")
                    nc.vector.tensor_tensor(
                        out=mus[:], in0=mv[:, 0:1], in1=scale_v[:],
                        op=mybir.AluOpType.mult)
                    shift_v = workp.tile([128, 1], F32, tag="shift_v")
                    nc.vector.tensor_tensor(
                        out=shift_v[:], in0=bnp_t[:, 4 * l + 2 + m:4 * l + 3 + m],
                        in1=mus[:], op=mybir.AluOpType.subtract)
                    for ns in range(8):
                        cs = slice(ns * 512, (ns + 1) * 512)
                        nc.scalar.activation(
                            out=xn[m][:, cs], in_=ysb[m][:, cs],
                            func=mybir.ActivationFunctionType.Relu,
                            bias=shift_v[:], scale=scale_v[:])
                xs = xn

            # ---- stage 4: logits z[p, j] = z_full[128*j + p]
            z_ps = psp1.tile([128, NB], F32, space="PSUM", tag="z")
            for j in range(NB):
                cs = slice(j * 128, (j + 1) * 128)
                nc.tensor.matmul(out=z_ps[:, j:j + 1], lhsT=xs[0][:, cs],
                                 rhs=wo_t[:, 0:1], start=True, stop=False)
                nc.tensor.matmul(out=z_ps[:, j:j + 1], lhsT=xs[1][:, cs],
                                 rhs=wo_t[:, 1:2], start=False, stop=True)
            z_sb = workp.tile([128, NB], F32, bufs=1)
            nc.scalar.activation(
                out=z_sb[:], in_=z_ps[:],
                func=mybir.ActivationFunctionType.Identity,
                bias=bnp_t[:, 12:13])
            nc.sync.dma_start(out=z_d[:], in_=z_sb[:])

            # ---- stage 5: BCE-with-logits, mean reduction
            relu_t = workp.tile([128, NB], F32, tag="bce1")
            nc.scalar.activation(out=relu_t[:], in_=z_sb[:],
                                 func=mybir.ActivationFunctionType.Relu)
            abs_t = workp.tile([128, NB], F32, tag="bce2")
            nc.scalar.activation(out=abs_t[:], in_=z_sb[:],
                                 func=mybir.ActivationFunctionType.Abs)
            e_t = workp.tile([128, NB], F32, tag="bce3a")
            nc.scalar.activation(out=e_t[:], in_=abs_t[:],
                                 func=mybir.ActivationFunctionType.Exp,
                                 scale=-1.0)
            sp_t = workp.tile([128, NB], F32, tag="bce3")
            nc.scalar.activation(out=sp_t[:], in_=e_t[:],
                                 func=mybir.ActivationFunctionType.Ln,
                                 bias=1.0)
            zt_t = workp.tile([128, NB], F32, tag="bce4")
            nc.vector.tensor_tensor(out=zt_t[:], in0=z_sb[:], in1=t_t[:],
                                    op=mybir.AluOpType.mult)
            s1_t = workp.tile([128, NB], F32, tag="bce5")
            nc.vector.tensor_tensor(out=s1_t[:], in0=relu_t[:], in1=sp_t[:],
                                    op=mybir.AluOpType.add)
            s2_t = workp.tile([128, NB], F32, tag="bce6")
            nc.vector.tensor_tensor(out=s2_t[:], in0=s1_t[:], in1=zt_t[:],
                                    op=mybir.AluOpType.subtract)
            red_t = workp.tile([128, 1], F32, tag="bce7")
            nc.vector.reduce_sum(out=red_t[:], in_=s2_t[:],
                                 axis=mybir.AxisListType.X)
            l_ps = psp1.tile([1, 1], F32, space="PSUM", tag="l")
            nc.tensor.matmul(out=l_ps[:], lhsT=red_t[:], rhs=ones_t[:],
                             start=True, stop=True)
            loss_sb = workp.tile([1, 1], F32, bufs=1)
            nc.scalar.activation(out=loss_sb[:], in_=l_ps[:],
                                 func=mybir.ActivationFunctionType.Identity,
                                 scale=1.0 / B)
            nc.sync.dma_start(out=loss_d[:], in_=loss_sb[:])

    nc.compile()
    return nc


# --------------------------------------------------------------------------
# entry point
# --------------------------------------------------------------------------

def kernel(token_ids, segment_ids, t, emb,
           W1, b1, g1, be1, W2, b2, g2, be2, W3, b3, g3, be3, Wo, bo,
           **_unused):
    token_ids = np.asarray(token_ids).astype(np.int64)
    segment_ids = np.asarray(segment_ids).astype(np.int64)
    t = np.asarray(t, dtype=np.float32)
    emb = np.ascontiguousarray(np.asarray(emb, dtype=np.float16))
    Ws = [np.ascontiguousarray(np.asarray(w, dtype=np.float32))
          for w in (W1, W2, W3)]
    Wo_np = np.ascontiguousarray(np.asarray(Wo, dtype=np.float32).reshape(H, 1))
    gs = [np.asarray(g, dtype=np.float32) for g in (g1, g2, g3)]
    bes = [np.asarray(be, dtype=np.float32) for be in (be1, be2, be3)]
    bo_np = np.asarray(bo, dtype=np.float32).reshape(-1)

    counts = np.bincount(segment_ids, minlength=B).astype(np.float32)
    invc_full = 1.0 / np.maximum(counts, 1.0)

    split = _split_tokens(token_ids, segment_ids)
    sched = _make_schedule(split)
    packed = [_pack_core(split[c], sched) for c in range(N_CORES)]
    idxcols = packed[0][0].shape[1]
    segocols = packed[0][1].shape[1]

    key = (tuple(tuple(s) for s in sched), idxcols, segocols)
    if key not in _PROGRAM_CACHE:
        _PROGRAM_CACHE.clear()
        _PROGRAM_CACHE[key] = _build_program(sched, idxcols, segocols)
    nc = _PROGRAM_CACHE[key]

    # shared input tensors
    iota = np.ascontiguousarray(np.broadcast_to(
        np.tile(np.arange(128, dtype=np.float32), 8), (128, 1024)))
    ident = np.eye(128, dtype=np.float32)
    tpk = np.ascontiguousarray(t.reshape(B // 128, 128).T)
    bnp = np.zeros((128, 16), np.float32)
    for l in range(3):
        for m in range(2):
            bnp[:, 4 * l + m] = gs[l][m * 128:(m + 1) * 128]
            bnp[:, 4 * l + 2 + m] = bes[l][m * 128:(m + 1) * 128]
    bnp[:, 12] = bo_np[0]
    bnp[:, 13] = BN_EPS

    in_maps = []
    for c in range(N_CORES):
        invc = np.ascontiguousarray(
            invc_full[c * SEGS_PER_CORE:(c + 1) * SEGS_PER_CORE]
            .reshape(BLOCKS, 128).T)
        m = {
            "emb": emb, "idx16": packed[c][0], "sego": packed[c][1],
            "invc": invc, "tpk": tpk, "wo": Wo_np, "bnp": bnp,
            "iota": iota, "ident": ident,
        }
        for l in range(3):
            m[f"w{l}"] = Ws[l]
        in_maps.append(m)

    global _LAST_IN_MAPS
    _LAST_IN_MAPS = in_maps
    res = run_bass_kernel_spmd(nc, in_maps, core_ids=list(range(N_CORES)))
    z_pk = res.results[0]["z"]
    loss = res.results[0]["loss"].reshape(())
    z = np.ascontiguousarray(z_pk.T.reshape(B)).astype(np.float32)
    return np.float32(loss), z


# revision 18
# speedup vs baseline: 1.5545x; 1.1154x over previous
"""Trainium2 Bass kernel for the ragged bag-of-words MLP model.

Strategy (data-parallel over 8 NeuronCores):
  - Each core owns 512 of the 4096 segments (batch rows). It gathers the
    embedding rows for its ~65536 tokens with `dma_gather` (custom GPSIMD
    ucode; int16 indices, so the 100k vocab is split into 4 contiguous
    buckets of <=32768 rows and tokens are bucket-sorted on the host).
  - Segment-sum is fused into the PE: for every 128-token chunk a one-hot
    selection matrix (seg-id vs iota, DVE is_equal) routes each token row
    to its segment row of a PSUM accumulator; division by counts is a
    per-partition tensor_scalar afterwards.
  - The per-core [512, 256] bag-of-words block is transposed (PE) to
    feature-major and AllGather'd so every core holds the full [256, 4096]
    activation; the 3-layer MLP + BatchNorm(train stats, via bn_stats /
    bn_aggr) + ReLU + logits + BCE loss are then computed redundantly on
    every core (no further collectives needed).
Outputs: loss [1,1] (same on every core) and z packed [128, 32]
(z_full[128*j + p] = z[p, j]); the host unpacks/concatenates.
"""

import sys

sys.path.insert(0, "/opt/trn_rl_repo")

import numpy as np

import concourse.bacc as bacc
import concourse.tile as tile
from concourse import mybir
from concourse.bass_utils import run_bass_kernel_spmd

F32 = mybir.dt.float32
F16 = mybir.dt.float16
I16 = mybir.dt.int16

VOCAB = 100000
H = 256
B = 4096
N_CORES = 8
SEGS_PER_CORE = B // N_CORES          # 512
BLOCKS = SEGS_PER_CORE // 128         # 4 blocks of 128 segments
BUCKET = 32768                        # int16-addressable vocab bucket
N_BUCKETS = (VOCAB + BUCKET - 1) // BUCKET  # 4
MAX_CHUNKS_PER_CALL = 32              # 4096 tokens / call
BN_EPS = 1e-5

_PROGRAM_CACHE: dict = {}


# --------------------------------------------------------------------------
# host-side packing
# --------------------------------------------------------------------------

def _split_tokens(token_ids, segment_ids):
    """Per core, per block, per bucket: (rel_idx int64 array, seg_local f32)."""
    out = []
    for c in range(N_CORES):
        seg0 = c * SEGS_PER_CORE
        lo = np.searchsorted(segment_ids, seg0, "left")
        hi = np.searchsorted(segment_ids, seg0 + SEGS_PER_CORE, "left")
        tok_c = token_ids[lo:hi]
        seg_c = segment_ids[lo:hi] - seg0
        blocks = []
        for b in range(BLOCKS):
            blo = np.searchsorted(seg_c, b * 128, "left")
            bhi = np.searchsorted(seg_c, (b + 1) * 128, "left")
            tok_b = tok_c[blo:bhi]
            seg_b = (seg_c[blo:bhi] - b * 128).astype(np.float32)
            buck = tok_b >> 15
            order = np.argsort(buck, kind="stable")
            tok_b, seg_b, buck = tok_b[order], seg_b[order], buck[order]
            per_bucket = []
            for r in range(N_BUCKETS):
                m = buck == r
                per_bucket.append((tok_b[m] - r * BUCKET, seg_b[m]))
            blocks.append(per_bucket)
        out.append(blocks)
    return out


def _make_schedule(split):
    """Unified call schedule: per block, list of (bucket, n_chunks<=16)."""
    sched = []
    for b in range(BLOCKS):
        calls = []
        for r in range(N_BUCKETS):
            nch = max(
                (len(split[c][b][r][0]) + 127) // 128 for c in range(N_CORES))
            pos = 0
            while pos < nch:
                take = min(MAX_CHUNKS_PER_CALL, nch - pos)
                calls.append((r, take))
                pos += take
        sched.append(calls)
    return sched


def _pack_core(split_c, sched):
    """Build idx16 [128, icols] and sego [128, ccols] for one core."""
    idx_cols, sego_cols = [], []
    for b in range(BLOCKS):
        consumed = {r: 0 for r in range(N_BUCKETS)}
        for (r, take) in sched[b]:
            tok, seg = split_c[b][r]
            s, n = consumed[r] * 128, take * 128
            ti = tok[s:s + n]
            si = seg[s:s + n]
            npad = n - len(ti)
            if npad:
                ti = np.concatenate([ti, np.zeros(npad, ti.dtype)])
                si = np.concatenate([si, np.full(npad, -1.0, np.float32)])
            # dma_gather index wrapping: index i -> [i % 16, i // 16],
            # replicated across the 8 groups of 16 partitions
            w = ti.astype(np.int16).reshape(n // 16, 16).T     # [16, n/16]
            idx_cols.append(np.tile(w, (8, 1)))
            # chunk j, lane p holds token i = j*128 + p
            sego_cols.append(si.reshape(take, 128).T)          # [128, take]
            consumed[r] += take
    idx16 = np.ascontiguousarray(np.concatenate(idx_cols, axis=1))
    sego = np.ascontiguousarray(np.concatenate(sego_cols, axis=1))
    return idx16, sego


# --------------------------------------------------------------------------
# device program
# --------------------------------------------------------------------------

def _build_program(schedule, idxcols, segocols):
    nc = bacc.Bacc("TRN2", target_bir_lowering=False, debug=False,
                   num_devices=N_CORES, num_swdge_queues=4)

    emb = nc.dram_tensor("emb", [VOCAB, H], F16, kind="ExternalInput")
    idx_d = nc.dram_tensor("idx16", [128, idxcols], I16, kind="ExternalInput")
    sego_d = nc.dram_tensor("sego", [128, segocols], F32, kind="ExternalInput")
    invc_d = nc.dram_tensor("invc", [128, BLOCKS], F32, kind="ExternalInput")
    t_d = nc.dram_tensor("tpk", [128, B // 128], F32, kind="ExternalInput")
    w_d = [nc.dram_tensor(f"w{l}", [H, H], F16, kind="ExternalInput")
           for l in range(3)]
    wo_d = nc.dram_tensor("wo", [H, 1], F16, kind="ExternalInput")
    bnp_d = nc.dram_tensor("bnp", [128, 16], F32, kind="ExternalInput")
    iota_d = nc.dram_tensor("iota", [128, 1024], F32, kind="ExternalInput")
    ident_d = nc.dram_tensor("ident", [128, 128], F16, kind="ExternalInput")
    z_d = nc.dram_tensor("z", [128, B // 128], F32, kind="ExternalOutput")
    loss_d = nc.dram_tensor("loss", [1, 1], F32, kind="ExternalOutput")

    NB = B // 128  # 32 batch column-chunks of 128

    with tile.TileContext(nc) as tc:
        with (
            tc.tile_pool(name="const", bufs=1) as constp,
            tc.tile_pool(name="gat", bufs=4) as gatp,
            tc.tile_pool(name="sel", bufs=4) as selp,
            tc.tile_pool(name="work", bufs=2) as workp,
            tc.tile_pool(name="act", bufs=2) as actp,
            tc.tile_pool(name="ybuf", bufs=1) as ybufp,
            tc.tile_pool(name="psum", bufs=2, space="PSUM") as psp,
            tc.tile_pool(name="psum1", bufs=1, space="PSUM") as psp1,
            tc.tile_pool(name="dram", bufs=1, space="DRAM") as dramp,
        ):
            # ---- constants / parameters into SBUF
            idx_t = constp.tile([128, idxcols], I16)
            nc.sync.dma_start(out=idx_t[:], in_=idx_d[:])
            sego_t = constp.tile([128, segocols], F32)
            nc.sync.dma_start(out=sego_t[:], in_=sego_d[:])
            invc_t = constp.tile([128, BLOCKS], F32)
            nc.sync.dma_start(out=invc_t[:], in_=invc_d[:])
            t_t = constp.tile([128, NB], F32)
            nc.sync.dma_start(out=t_t[:], in_=t_d[:])
            iota_t = constp.tile([128, 1024], F32)
            nc.sync.dma_start(out=iota_t[:], in_=iota_d[:])
            ident_t = constp.tile([128, 128], F16)
            nc.sync.dma_start(out=ident_t[:], in_=ident_d[:])
            bnp_t = constp.tile([128, 16], F32)
            nc.sync.dma_start(out=bnp_t[:], in_=bnp_d[:])
            w_t = []
            for l in range(3):
                wa = constp.tile([128, H], F16, name=f"w{l}a")
                nc.sync.dma_start(out=wa[:], in_=w_d[l][0:128, :])
                wb = constp.tile([128, H], F16, name=f"w{l}b")
                nc.sync.dma_start(out=wb[:], in_=w_d[l][128:256, :])
                w_t.append((wa, wb))
            wo_t = constp.tile([128, 2], F16)
            nc.sync.dma_start(out=wo_t[:, 0:1], in_=wo_d[0:128, :])
            nc.sync.dma_start(out=wo_t[:, 1:2], in_=wo_d[128:256, :])
            ones_t = constp.tile([128, 1], F32)
            nc.vector.memset(ones_t[:], 1.0)

            # ---- stage 1: gather + segment-mean, per block of 128 segments
            xsh = [workp.tile([128, SEGS_PER_CORE], F16, name=f"xsh{h}", bufs=1)
                   for h in range(2)]  # feature-major bow shard
            icol = 0   # running idx16 column offset (n_idxs/16 per call)
            ccol = 0   # running chunk column offset
            call_no = 0
            for b in range(BLOCKS):
                calls = schedule[b]
                nch_total = sum(c[1] for c in calls)
                bow_ps = psp.tile([128, H], F32, space="PSUM", tag="bow")
                chunk = 0
                for (r, take) in calls:
                    n_i = take * 128
                    gat = gatp.tile([128, take, H], F16, tag="gat")
                    base = r * BUCKET
                    rows = min(BUCKET, VOCAB - base)
                    nc.gpsimd.dma_gather(
                        gat[:],
                        emb[base:base + rows, :],
                        idx_t[:, icol:icol + n_i // 16],
                        n_i,
                        n_i,
                        H,
                        single_packet=False,
                        queue_num=call_no % 4,
                    )
                    call_no += 1
                    icol += n_i // 16
                    for g0 in range(0, take, 8):
                        gn = min(8, take - g0)
                        sel8 = selp.tile([128, 8, 128], F16, tag="sel")
                        nc.vector.tensor_tensor(
                            out=sel8[:, 0:gn, :],
                            in0=sego_t[:, ccol + g0:ccol + g0 + gn]
                            .rearrange("p (c o) -> p c o", o=1)
                            .to_broadcast([128, gn, 128]),
                            in1=iota_t[:, 0:gn * 128]
                            .rearrange("p (c i) -> p c i", c=gn),
                            op=mybir.AluOpType.is_equal,
                        )
                        for jj in range(gn):
                            nc.tensor.matmul(
                                out=bow_ps[:],
                                lhsT=sel8[:, jj, :],
                                rhs=gat[:, g0 + jj, :],
                                start=(chunk == 0),
                                stop=(chunk == nch_total - 1),
                            )
                            chunk += 1
                    ccol += take
                # mean + transpose to feature-major shard columns
                bow_sb = workp.tile([128, H], F16, tag="bow_sb")
                nc.vector.tensor_scalar(
                    out=bow_sb[:], in0=bow_ps[:],
                    scalar1=invc_t[:, b:b + 1], scalar2=None,
                    op0=mybir.AluOpType.mult,
                )
                for h in range(2):
                    tp_ps = psp.tile([128, 128], F16, space="PSUM", tag="tp")
                    nc.tensor.transpose(
                        out=tp_ps[:], in_=bow_sb[:, h * 128:(h + 1) * 128],
                        identity=ident_t[:],
                    )
                    nc.vector.tensor_copy(
                        out=xsh[h][:, b * 128:(b + 1) * 128], in_=tp_ps[:])

            # ---- stage 2: AllGather bow across the 8 cores
            ag_in = dramp.tile([2, 128, SEGS_PER_CORE], F16)
            for h in range(2):
                nc.sync.dma_start(out=ag_in[h], in_=xsh[h][:])
            ag_out = dramp.tile([N_CORES, 2, 128, SEGS_PER_CORE], F16,
                                addr_space="Shared")
            nc.gpsimd.collective_compute(
                "AllGather",
                mybir.AluOpType.bypass,
                replica_groups=[list(range(N_CORES))],
                ins=[ag_in.opt()],
                outs=[ag_out.opt()],
            )
            x0 = actp.tile([128, B], F16, name="x0", tag="actH0")
            x1 = actp.tile([128, B], F16, name="x1", tag="actH1")
            for r in range(N_CORES):
                s = slice(r * SEGS_PER_CORE, (r + 1) * SEGS_PER_CORE)
                nc.sync.dma_start(out=x0[:, s], in_=ag_out[r, 0])
                nc.sync.dma_start(out=x1[:, s], in_=ag_out[r, 1])

            # ---- stage 3: MLP with BatchNorm(training stats) + ReLU
            xs = (x0, x1)
            for l in range(3):
                wa, wb = w_t[l]
                ysb = [ybufp.tile([128, B], F16, name=f"y{l}{m}", tag=f"ysb{m}")
                       for m in range(2)]
                stats = workp.tile([128, 2, 8, 6], F32, tag="stats")
                for m in range(2):
                    for ns in range(8):
                        cs = slice(ns * 512, (ns + 1) * 512)
                        y_ps = psp.tile([128, 512], F32, space="PSUM", tag="y")
                        nc.tensor.matmul(
                            out=y_ps[:], lhsT=wa[:, m * 128:(m + 1) * 128],
                            rhs=xs[0][:, cs], start=True, stop=False)
                        nc.tensor.matmul(
                            out=y_ps[:], lhsT=wb[:, m * 128:(m + 1) * 128],
                            rhs=xs[1][:, cs], start=False, stop=True)
                        nc.vector.tensor_copy(out=ysb[m][:, cs], in_=y_ps[:])
                        nc.vector.bn_stats(
                            out=stats[:, m, ns, :], in_=ysb[m][:, cs])
                xn = [actp.tile([128, B], F16, name=f"xn{l}{m}", tag=f"actH{m}")
                      for m in range(2)]
                for m in range(2):
                    mv = workp.tile([128, 2], F32, tag="mv")
                    nc.vector.bn_aggr(out=mv[:], in_=stats[:, m, :, :])
                    std = workp.tile([128, 1], F32, tag="std")
                    nc.scalar.activation(
                        out=std[:], in_=mv[:, 1:2],
                        func=mybir.ActivationFunctionType.Sqrt,
                        bias=bnp_t[:, 13:14])
                    rstd = workp.tile([128, 1], F32, tag="rstd")
                    nc.vector.reciprocal(out=rstd[:], in_=std[:])
                    scale_v = workp.tile([128, 1], F32, tag="scale_v")
                    nc.vector.tensor_tensor(
                        out=scale_v[:], in0=rstd[:],
                        in1=bnp_t[:, 4 * l + m:4 * l + m + 1],
                        op=mybir.AluOpType.mult)
                    mus = workp.tile([128, 1], F32, tag="mus")
                    nc.vector.tensor_tensor(
                        out=mus[:], in0=mv[:, 0:1], in1=scale_v[:],
                        op=mybir.AluOpType.mult)
                    shift_v = workp.tile([128, 1], F32, tag="shift_v")
                    nc.vector.tensor_tensor(
                        out=shift_v[:], in0=bnp_t[:, 4 * l + 2 + m:4 * l + 3 + m],
                        in1=mus[:], op=mybir.AluOpType.subtract)
                    for ns in range(8):
                        cs = slice(ns * 512, (ns + 1) * 512)
                        nc.scalar.activation(
                            out=xn[m][:, cs], in_=ysb[m][:, cs],
                            func=mybir.ActivationFunctionType.Relu,
                            bias=shift_v[:], scale=scale_v[:])
                xs = xn

            # ---- stage 4: logits z[p, j] = z_full[128*j + p]
            z_ps = psp1.tile([128, NB], F32, space="PSUM", tag="z")
            for j in range(NB):
                cs = slice(j * 128, (j + 1) * 128)
                nc.tensor.matmul(out=z_ps[:, j:j + 1], lhsT=xs[0][:, cs],
                                 rhs=wo_t[:, 0:1], start=True, stop=False)
                nc.tensor.matmul(out=z_ps[:, j:j + 1], lhsT=xs[1][:, cs],
                                 rhs=wo_t[:, 1:2], start=False, stop=True)
            z_sb = workp.tile([128, NB], F32, bufs=1)
            nc.scalar.activation(
                out=z_sb[:], in_=z_ps[:],
                func=mybir.ActivationFunctionType.Identity,
                bias=bnp_t[:, 12:13])
            nc.sync.dma_start(out=z_d[:], in_=z_sb[:])

            # ---- stage 5: BCE-with-logits, mean reduction
            relu_t = workp.tile([128, NB], F32, tag="bce1")
            nc.scalar.activation(out=relu_t[:], in_=z_sb[:],
                                 func=mybir.ActivationFunctionType.Relu)
            abs_t = workp.tile([128, NB], F32, tag="bce2")
            nc.scalar.activation(out=abs_t[:], in_=z_sb[:],
                                 func=mybir.ActivationFunctionType.Abs)
            e_t = workp.tile([128, NB], F32, tag="bce3a")
            nc.scalar.activation(out=e_t[:], in_=abs_t[:],
                                 func=mybir.ActivationFunctionType.Exp,
                                 scale=-1.0)
            sp_t = workp.tile([128, NB], F32, tag="bce3")
            nc.scalar.activation(out=sp_t[:], in_=e_t[:],
                                 func=mybir.ActivationFunctionType.Ln,
                                 bias=1.0)
            zt_t = workp.tile([128, NB], F32, tag="bce4")
            nc.vector.tensor_tensor(out=zt_t[:], in0=z_sb[:], in1=t_t[:],
                                    op=mybir.AluOpType.mult)
            s1_t = workp.tile([128, NB], F32, tag="bce5")
            nc.vector.tensor_tensor(out=s1_t[:], in0=relu_t[:], in1=sp_t[:],
                                    op=mybir.AluOpType.add)
            s2_t = workp.tile([128, NB], F32, tag="bce6")
            nc.vector.tensor_tensor(out=s2_t[:], in0=s1_t[:], in1=zt_t[:],
                                    op=mybir.AluOpType.subtract)
            red_t = workp.tile([128, 1], F32, tag="bce7")
            nc.vector.reduce_sum(out=red_t[:], in_=s2_t[:],
                                 axis=mybir.AxisListType.X)
            l_ps = psp1.tile([1, 1], F32, space="PSUM", tag="l")
            nc.tensor.matmul(out=l_ps[:], lhsT=red_t[:], rhs=ones_t[:],
                             start=True, stop=True)
            loss_sb = workp.tile([1, 1], F32, bufs=1)
            nc.scalar.activation(out=loss_sb[:], in_=l_ps[:],
                                 func=mybir.ActivationFunctionType.Identity,
                                 scale=1.0 / B)
            nc.sync.dma_start(out=loss_d[:], in_=loss_sb[:])

    nc.compile()
    return nc


# --------------------------------------------------------------------------
# entry point
# --------------------------------------------------------------------------

def kernel(token_ids, segment_ids, t, emb,
           W1, b1, g1, be1, W2, b2, g2, be2, W3, b3, g3, be3, Wo, bo,
           **_unused):
    token_ids = np.asarray(token_ids).astype(np.int64)
    segment_ids = np.asarray(segment_ids).astype(np.int64)
    t = np.asarray(t, dtype=np.float32)
    emb = np.ascontiguousarray(np.asarray(emb, dtype=np.float16))
    Ws = [np.ascontiguousarray(np.asarray(w, dtype=np.float16))
          for w in (W1, W2, W3)]
    Wo_np = np.ascontiguousarray(np.asarray(Wo, dtype=np.float16).reshape(H, 1))
    gs = [np.asarray(g, dtype=np.float32) for g in (g1, g2, g3)]
    bes = [np.asarray(be, dtype=np.float32) for be in (be1, be2, be3)]
    bo_np = np.asarray(bo, dtype=np.float32).reshape(-1)

    counts = np.bincount(segment_ids, minlength=B).astype(np.float32)
    invc_full = 1.0 / np.maximum(counts, 1.0)

    split = _split_tokens(token_ids, segment_ids)
    sched = _make_schedule(split)
    packed = [_pack_core(split[c], sched) for c in range(N_CORES)]
    idxcols = packed[0][0].shape[1]
    segocols = packed[0][1].shape[1]

    key = (tuple(tuple(s) for s in sched), idxcols, segocols)
    if key not in _PROGRAM_CACHE:
        _PROGRAM_CACHE.clear()
        _PROGRAM_CACHE[key] = _build_program(sched, idxcols, segocols)
    nc = _PROGRAM_CACHE[key]

    # shared input tensors
    iota = np.ascontiguousarray(np.broadcast_to(
        np.tile(np.arange(128, dtype=np.float32), 8), (128, 1024)))
    ident = np.eye(128, dtype=np.float16)
    tpk = np.ascontiguousarray(t.reshape(B // 128, 128).T)
    bnp = np.zeros((128, 16), np.float32)
    for l in range(3):
        for m in range(2):
            bnp[:, 4 * l + m] = gs[l][m * 128:(m + 1) * 128]
            bnp[:, 4 * l + 2 + m] = bes[l][m * 128:(m + 1) * 128]
    bnp[:, 12] = bo_np[0]
    bnp[:, 13] = BN_EPS

    in_maps = []
    for c in range(N_CORES):
        invc = np.ascontiguousarray(
            invc_full[c * SEGS_PER_CORE:(c + 1) * SEGS_PER_CORE]
            .reshape(BLOCKS, 128).T)
        m = {
            "emb": emb, "idx16": packed[c][0], "sego": packed[c][1],
            "invc": invc, "tpk": tpk, "wo": Wo_np, "bnp": bnp,
            "iota": iota, "ident": ident,
        }
        for l in range(3):
            m[f"w{l}"] = Ws[l]
        in_maps.append(m)

    global _LAST_IN_MAPS
    _LAST_IN_MAPS = in_maps
    res = run_bass_kernel_spmd(nc, in_maps, core_ids=list(range(N_CORES)))
    z_pk = res.results[0]["z"]
    loss = res.results[0]["loss"].reshape(())
    z = np.ascontiguousarray(z_pk.T.reshape(B)).astype(np.float32)
    return np.float32(loss), z


# revision 19
# speedup vs baseline: 1.7409x; 1.1199x over previous
"""Trainium2 Bass kernel for the ragged bag-of-words MLP model.

Strategy (data-parallel over 8 NeuronCores):
  - Each core owns 512 of the 4096 segments (batch rows). It gathers the
    embedding rows for its ~65536 tokens with `dma_gather` (custom GPSIMD
    ucode; int16 indices, so the 100k vocab is split into 4 contiguous
    buckets of <=32768 rows and tokens are bucket-sorted on the host).
  - Segment-sum is fused into the PE: for every 128-token chunk a one-hot
    selection matrix (seg-id vs iota, DVE is_equal) routes each token row
    to its segment row of a PSUM accumulator; division by counts is a
    per-partition tensor_scalar afterwards.
  - The per-core [512, 256] bag-of-words block is transposed (PE) to
    feature-major and AllGather'd so every core holds the full [256, 4096]
    activation; the 3-layer MLP + BatchNorm(train stats, via bn_stats /
    bn_aggr) + ReLU + logits + BCE loss are then computed redundantly on
    every core (no further collectives needed).
Outputs: loss [1,1] (same on every core) and z packed [128, 32]
(z_full[128*j + p] = z[p, j]); the host unpacks/concatenates.
"""

import sys

sys.path.insert(0, "/opt/trn_rl_repo")

import numpy as np

import concourse.bacc as bacc
import concourse.tile as tile
from concourse import mybir
from concourse.bass_utils import run_bass_kernel_spmd

F32 = mybir.dt.float32
F16 = mybir.dt.float16
I16 = mybir.dt.int16

VOCAB = 100000
H = 256
B = 4096
N_CORES = 8
SEGS_PER_CORE = B // N_CORES          # 512
BLOCKS = SEGS_PER_CORE // 128         # 4 blocks of 128 segments
BUCKET = 32768                        # int16-addressable vocab bucket
N_BUCKETS = (VOCAB + BUCKET - 1) // BUCKET  # 4
MAX_CHUNKS_PER_CALL = 32              # 4096 tokens / call
BN_EPS = 1e-5

_PROGRAM_CACHE: dict = {}


# --------------------------------------------------------------------------
# host-side packing
# --------------------------------------------------------------------------

def _split_tokens(token_ids, segment_ids):
    """Per core, per block, per bucket: (rel_idx int64 array, seg_local f32)."""
    out = []
    for c in range(N_CORES):
        seg0 = c * SEGS_PER_CORE
        lo = np.searchsorted(segment_ids, seg0, "left")
        hi = np.searchsorted(segment_ids, seg0 + SEGS_PER_CORE, "left")
        tok_c = token_ids[lo:hi]
        seg_c = segment_ids[lo:hi] - seg0
        blocks = []
        for b in range(BLOCKS):
            blo = np.searchsorted(seg_c, b * 128, "left")
            bhi = np.searchsorted(seg_c, (b + 1) * 128, "left")
            tok_b = tok_c[blo:bhi]
            seg_b = (seg_c[blo:bhi] - b * 128).astype(np.float32)
            buck = tok_b >> 15
            order = np.argsort(buck, kind="stable")
            tok_b, seg_b, buck = tok_b[order], seg_b[order], buck[order]
            per_bucket = []
            for r in range(N_BUCKETS):
                m = buck == r
                per_bucket.append((tok_b[m] - r * BUCKET, seg_b[m]))
            blocks.append(per_bucket)
        out.append(blocks)
    return out


def _make_schedule(split):
    """Unified call schedule: per block, list of (bucket, n_chunks<=16)."""
    sched = []
    for b in range(BLOCKS):
        calls = []
        for r in range(N_BUCKETS):
            nch = max(
                (len(split[c][b][r][0]) + 127) // 128 for c in range(N_CORES))
            pos = 0
            while pos < nch:
                take = min(MAX_CHUNKS_PER_CALL, nch - pos)
                calls.append((r, take))
                pos += take
        sched.append(calls)
    return sched


def _pack_core(split_c, sched):
    """Build idx16 [128, icols] and sego [128, ccols] for one core."""
    idx_cols, sego_cols = [], []
    for b in range(BLOCKS):
        consumed = {r: 0 for r in range(N_BUCKETS)}
        for (r, take) in sched[b]:
            tok, seg = split_c[b][r]
            s, n = consumed[r] * 128, take * 128
            ti = tok[s:s + n]
            si = seg[s:s + n]
            npad = n - len(ti)
            if npad:
                ti = np.concatenate([ti, np.zeros(npad, ti.dtype)])
                si = np.concatenate([si, np.full(npad, -1.0, np.float32)])
            # dma_gather index wrapping: index i -> [i % 16, i // 16],
            # replicated across the 8 groups of 16 partitions
            w = ti.astype(np.int16).reshape(n // 16, 16).T     # [16, n/16]
            idx_cols.append(np.tile(w, (8, 1)))
            # chunk j, lane p holds token i = j*128 + p
            sego_cols.append(si.reshape(take, 128).T)          # [128, take]
            consumed[r] += take
    idx16 = np.ascontiguousarray(np.concatenate(idx_cols, axis=1))
    sego = np.ascontiguousarray(np.concatenate(sego_cols, axis=1))
    return idx16, sego


# --------------------------------------------------------------------------
# device program
# --------------------------------------------------------------------------

def _build_program(schedule, idxcols, segocols):
    nc = bacc.Bacc("TRN2", target_bir_lowering=False, debug=False,
                   num_devices=N_CORES, num_swdge_queues=4)

    emb = nc.dram_tensor("emb", [VOCAB, H], F16, kind="ExternalInput")
    idx_d = nc.dram_tensor("idx16", [128, idxcols], I16, kind="ExternalInput")
    sego_d = nc.dram_tensor("sego", [128, segocols], F32, kind="ExternalInput")
    invc_d = nc.dram_tensor("invc", [128, BLOCKS], F32, kind="ExternalInput")
    t_d = nc.dram_tensor("tpk", [128, B // 128], F32, kind="ExternalInput")
    w_d = [nc.dram_tensor(f"w{l}", [H, H], F16, kind="ExternalInput")
           for l in range(3)]
    wo_d = nc.dram_tensor("wo", [H, 1], F16, kind="ExternalInput")
    bnp_d = nc.dram_tensor("bnp", [128, 16], F32, kind="ExternalInput")
    iota_d = nc.dram_tensor("iota", [128, 1024], F32, kind="ExternalInput")
    ident_d = nc.dram_tensor("ident", [128, 128], F16, kind="ExternalInput")
    z_d = nc.dram_tensor("z", [128, B // 128], F32, kind="ExternalOutput")
    loss_d = nc.dram_tensor("loss", [1, 1], F32, kind="ExternalOutput")

    NB = B // 128  # 32 batch column-chunks of 128

    with tile.TileContext(nc) as tc:
        with (
            tc.tile_pool(name="const", bufs=1) as constp,
            tc.tile_pool(name="gat", bufs=6) as gatp,
            tc.tile_pool(name="sel", bufs=6) as selp,
            tc.tile_pool(name="work", bufs=2) as workp,
            tc.tile_pool(name="act", bufs=2) as actp,
            tc.tile_pool(name="ybuf", bufs=1) as ybufp,
            tc.tile_pool(name="psum", bufs=2, space="PSUM") as psp,
            tc.tile_pool(name="psum1", bufs=1, space="PSUM") as psp1,
            tc.tile_pool(name="dram", bufs=1, space="DRAM") as dramp,
        ):
            # ---- constants / parameters into SBUF
            idx_t = constp.tile([128, idxcols], I16)
            nc.sync.dma_start(out=idx_t[:], in_=idx_d[:])
            sego_t = constp.tile([128, segocols], F32)
            nc.sync.dma_start(out=sego_t[:], in_=sego_d[:])
            invc_t = constp.tile([128, BLOCKS], F32)
            nc.sync.dma_start(out=invc_t[:], in_=invc_d[:])
            t_t = constp.tile([128, NB], F32)
            nc.sync.dma_start(out=t_t[:], in_=t_d[:])
            iota_t = constp.tile([128, 1024], F32)
            nc.sync.dma_start(out=iota_t[:], in_=iota_d[:])
            ident_t = constp.tile([128, 128], F16)
            nc.sync.dma_start(out=ident_t[:], in_=ident_d[:])
            bnp_t = constp.tile([128, 16], F32)
            nc.sync.dma_start(out=bnp_t[:], in_=bnp_d[:])
            w_t = []
            for l in range(3):
                wa = constp.tile([128, H], F16, name=f"w{l}a")
                nc.sync.dma_start(out=wa[:], in_=w_d[l][0:128, :])
                wb = constp.tile([128, H], F16, name=f"w{l}b")
                nc.sync.dma_start(out=wb[:], in_=w_d[l][128:256, :])
                w_t.append((wa, wb))
            wo_t = constp.tile([128, 2], F16)
            nc.sync.dma_start(out=wo_t[:, 0:1], in_=wo_d[0:128, :])
            nc.sync.dma_start(out=wo_t[:, 1:2], in_=wo_d[128:256, :])
            ones_t = constp.tile([128, 1], F32)
            nc.vector.memset(ones_t[:], 1.0)

            # ---- stage 1: gather + segment-mean, per block of 128 segments
            xsh = [workp.tile([128, SEGS_PER_CORE], F16, name=f"xsh{h}", bufs=1)
                   for h in range(2)]  # feature-major bow shard
            icol = 0   # running idx16 column offset (n_idxs/16 per call)
            ccol = 0   # running chunk column offset
            call_no = 0
            for b in range(BLOCKS):
                calls = schedule[b]
                nch_total = sum(c[1] for c in calls)
                bow_ps = psp.tile([128, H], F32, space="PSUM", tag="bow")
                chunk = 0
                for (r, take) in calls:
                    n_i = take * 128
                    gat = gatp.tile([128, take, H], F16, tag="gat")
                    base = r * BUCKET
                    rows = min(BUCKET, VOCAB - base)
                    nc.gpsimd.dma_gather(
                        gat[:],
                        emb[base:base + rows, :],
                        idx_t[:, icol:icol + n_i // 16],
                        n_i,
                        n_i,
                        H,
                        single_packet=False,
                        queue_num=call_no % 4,
                    )
                    call_no += 1
                    icol += n_i // 16
                    for g0 in range(0, take, 8):
                        gn = min(8, take - g0)
                        sel8 = selp.tile([128, 8, 128], F16, tag="sel")
                        nc.vector.tensor_tensor(
                            out=sel8[:, 0:gn, :],
                            in0=sego_t[:, ccol + g0:ccol + g0 + gn]
                            .rearrange("p (c o) -> p c o", o=1)
                            .to_broadcast([128, gn, 128]),
                            in1=iota_t[:, 0:gn * 128]
                            .rearrange("p (c i) -> p c i", c=gn),
                            op=mybir.AluOpType.is_equal,
                        )
                        for jj in range(gn):
                            nc.tensor.matmul(
                                out=bow_ps[:],
                                lhsT=sel8[:, jj, :],
                                rhs=gat[:, g0 + jj, :],
                                start=(chunk == 0),
                                stop=(chunk == nch_total - 1),
                            )
                            chunk += 1
                    ccol += take
                # mean + transpose to feature-major shard columns
                bow_sb = workp.tile([128, H], F16, tag="bow_sb")
                nc.vector.tensor_scalar(
                    out=bow_sb[:], in0=bow_ps[:],
                    scalar1=invc_t[:, b:b + 1], scalar2=None,
                    op0=mybir.AluOpType.mult,
                )
                for h in range(2):
                    tp_ps = psp.tile([128, 128], F16, space="PSUM", tag="tp")
                    nc.tensor.transpose(
                        out=tp_ps[:], in_=bow_sb[:, h * 128:(h + 1) * 128],
                        identity=ident_t[:],
                    )
                    nc.vector.tensor_copy(
                        out=xsh[h][:, b * 128:(b + 1) * 128], in_=tp_ps[:])

            # ---- stage 2: AllGather bow across the 8 cores
            ag_in = dramp.tile([2, 128, SEGS_PER_CORE], F16)
            for h in range(2):
                nc.sync.dma_start(out=ag_in[h], in_=xsh[h][:])
            ag_out = dramp.tile([N_CORES, 2, 128, SEGS_PER_CORE], F16,
                                addr_space="Shared")
            nc.gpsimd.collective_compute(
                "AllGather",
                mybir.AluOpType.bypass,
                replica_groups=[list(range(N_CORES))],
                ins=[ag_in.opt()],
                outs=[ag_out.opt()],
            )
            x0 = actp.tile([128, B], F16, name="x0", tag="actH0")
            x1 = actp.tile([128, B], F16, name="x1", tag="actH1")
            for r in range(N_CORES):
                s = slice(r * SEGS_PER_CORE, (r + 1) * SEGS_PER_CORE)
                nc.sync.dma_start(out=x0[:, s], in_=ag_out[r, 0])
                nc.sync.dma_start(out=x1[:, s], in_=ag_out[r, 1])

            # ---- stage 3: MLP with BatchNorm(training stats) + ReLU
            xs = (x0, x1)
            for l in range(3):
                wa, wb = w_t[l]
                ysb = [ybufp.tile([128, B], F16, name=f"y{l}{m}", tag=f"ysb{m}")
                       for m in range(2)]
                stats = workp.tile([128, 2, 8, 6], F32, tag="stats")
                for m in range(2):
                    for ns in range(8):
                        cs = slice(ns * 512, (ns + 1) * 512)
                        y_ps = psp.tile([128, 512], F32, space="PSUM", tag="y")
                        nc.tensor.matmul(
                            out=y_ps[:], lhsT=wa[:, m * 128:(m + 1) * 128],
                            rhs=xs[0][:, cs], start=True, stop=False)
                        nc.tensor.matmul(
                            out=y_ps[:], lhsT=wb[:, m * 128:(m + 1) * 128],
                            rhs=xs[1][:, cs], start=False, stop=True)
                        nc.scalar.copy(out=ysb[m][:, cs], in_=y_ps[:])
                        nc.vector.bn_stats(
                            out=stats[:, m, ns, :], in_=y_ps[:])
                xn = [actp.tile([128, B], F16, name=f"xn{l}{m}", tag=f"actH{m}")
                      for m in range(2)]
                for m in range(2):
                    mv = workp.tile([128, 2], F32, tag="mv")
                    nc.vector.bn_aggr(out=mv[:], in_=stats[:, m, :, :])
                    std = workp.tile([128, 1], F32, tag="std")
                    nc.scalar.activation(
                        out=std[:], in_=mv[:, 1:2],
                        func=mybir.ActivationFunctionType.Sqrt,
                        bias=bnp_t[:, 13:14])
                    rstd = workp.tile([128, 1], F32, tag="rstd")
                    nc.vector.reciprocal(out=rstd[:], in_=std[:])
                    scale_v = workp.tile([128, 1], F32, tag="scale_v")
                    nc.vector.tensor_tensor(
                        out=scale_v[:], in0=rstd[:],
                        in1=bnp_t[:, 4 * l + m:4 * l + m + 1],
                        op=mybir.AluOpType.mult)
                    mus = workp.tile([128, 1], F32, tag="mus")
                    nc.vector.tensor_tensor(
                        out=mus[:], in0=mv[:, 0:1], in1=scale_v[:],
                        op=mybir.AluOpType.mult)
                    shift_v = workp.tile([128, 1], F32, tag="shift_v")
                    nc.vector.tensor_tensor(
                        out=shift_v[:], in0=bnp_t[:, 4 * l + 2 + m:4 * l + 3 + m],
                        in1=mus[:], op=mybir.AluOpType.subtract)
                    for ns in range(8):
                        cs = slice(ns * 512, (ns + 1) * 512)
                        nc.scalar.activation(
                            out=xn[m][:, cs], in_=ysb[m][:, cs],
                            func=mybir.ActivationFunctionType.Relu,
                            bias=shift_v[:], scale=scale_v[:])
                xs = xn

            # ---- stage 4: logits z[p, j] = z_full[128*j + p]
            z_ps = psp1.tile([128, NB], F32, space="PSUM", tag="z")
            for j in range(NB):
                cs = slice(j * 128, (j + 1) * 128)
                nc.tensor.matmul(out=z_ps[:, j:j + 1], lhsT=xs[0][:, cs],
                                 rhs=wo_t[:, 0:1], start=True, stop=False)
                nc.tensor.matmul(out=z_ps[:, j:j + 1], lhsT=xs[1][:, cs],
                                 rhs=wo_t[:, 1:2], start=False, stop=True)
            z_sb = workp.tile([128, NB], F32, bufs=1)
            nc.scalar.activation(
                out=z_sb[:], in_=z_ps[:],
                func=mybir.ActivationFunctionType.Identity,
                bias=bnp_t[:, 12:13])
            nc.sync.dma_start(out=z_d[:], in_=z_sb[:])

            # ---- stage 5: BCE-with-logits, mean reduction
            relu_t = workp.tile([128, NB], F32, tag="bce1")
            nc.scalar.activation(out=relu_t[:], in_=z_sb[:],
                                 func=mybir.ActivationFunctionType.Relu)
            abs_t = workp.tile([128, NB], F32, tag="bce2")
            nc.scalar.activation(out=abs_t[:], in_=z_sb[:],
                                 func=mybir.ActivationFunctionType.Abs)
            e_t = workp.tile([128, NB], F32, tag="bce3a")
            nc.scalar.activation(out=e_t[:], in_=abs_t[:],
                                 func=mybir.ActivationFunctionType.Exp,
                                 scale=-1.0)
            sp_t = workp.tile([128, NB], F32, tag="bce3")
            nc.scalar.activation(out=sp_t[:], in_=e_t[:],
                                 func=mybir.ActivationFunctionType.Ln,
                                 bias=1.0)
            zt_t = workp.tile([128, NB], F32, tag="bce4")
            nc.vector.tensor_tensor(out=zt_t[:], in0=z_sb[:], in1=t_t[:],
                                    op=mybir.AluOpType.mult)
            s1_t = workp.tile([128, NB], F32, tag="bce5")
            nc.vector.tensor_tensor(out=s1_t[:], in0=relu_t[:], in1=sp_t[:],
                                    op=mybir.AluOpType.add)
            s2_t = workp.tile([128, NB], F32, tag="bce6")
            nc.vector.tensor_tensor(out=s2_t[:], in0=s1_t[:], in1=zt_t[:],
                                    op=mybir.AluOpType.subtract)
            red_t = workp.tile([128, 1], F32, tag="bce7")
            nc.vector.reduce_sum(out=red_t[:], in_=s2_t[:],
                                 axis=mybir.AxisListType.X)
            l_ps = psp1.tile([1, 1], F32, space="PSUM", tag="l")
            nc.tensor.matmul(out=l_ps[:], lhsT=red_t[:], rhs=ones_t[:],
                             start=True, stop=True)
            loss_sb = workp.tile([1, 1], F32, bufs=1)
            nc.scalar.activation(out=loss_sb[:], in_=l_ps[:],
                                 func=mybir.ActivationFunctionType.Identity,
                                 scale=1.0 / B)
            nc.sync.dma_start(out=loss_d[:], in_=loss_sb[:])

    nc.compile()
    return nc


# --------------------------------------------------------------------------
# entry point
# --------------------------------------------------------------------------

def kernel(token_ids, segment_ids, t, emb,
           W1, b1, g1, be1, W2, b2, g2, be2, W3, b3, g3, be3, Wo, bo,
           **_unused):
    token_ids = np.asarray(token_ids).astype(np.int64)
    segment_ids = np.asarray(segment_ids).astype(np.int64)
    t = np.asarray(t, dtype=np.float32)
    emb = np.ascontiguousarray(np.asarray(emb, dtype=np.float16))
    Ws = [np.ascontiguousarray(np.asarray(w, dtype=np.float16))
          for w in (W1, W2, W3)]
    Wo_np = np.ascontiguousarray(np.asarray(Wo, dtype=np.float16).reshape(H, 1))
    gs = [np.asarray(g, dtype=np.float32) for g in (g1, g2, g3)]
    bes = [np.asarray(be, dtype=np.float32) for be in (be1, be2, be3)]
    bo_np = np.asarray(bo, dtype=np.float32).reshape(-1)

    counts = np.bincount(segment_ids, minlength=B).astype(np.float32)
    invc_full = 1.0 / np.maximum(counts, 1.0)

    split = _split_tokens(token_ids, segment_ids)
    sched = _make_schedule(split)
    packed = [_pack_core(split[c], sched) for c in range(N_CORES)]
    idxcols = packed[0][0].shape[1]
    segocols = packed[0][1].shape[1]

    key = (tuple(tuple(s) for s in sched), idxcols, segocols)
    if key not in _PROGRAM_CACHE:
        _PROGRAM_CACHE.clear()
        _PROGRAM_CACHE[key] = _build_program(sched, idxcols, segocols)
    nc = _PROGRAM_CACHE[key]

    # shared input tensors
    iota = np.ascontiguousarray(np.broadcast_to(
        np.tile(np.arange(128, dtype=np.float32), 8), (128, 1024)))
    ident = np.eye(128, dtype=np.float16)
    tpk = np.ascontiguousarray(t.reshape(B // 128, 128).T)
    bnp = np.zeros((128, 16), np.float32)
    for l in range(3):
        for m in range(2):
            bnp[:, 4 * l + m] = gs[l][m * 128:(m + 1) * 128]
            bnp[:, 4 * l + 2 + m] = bes[l][m * 128:(m + 1) * 128]
    bnp[:, 12] = bo_np[0]
    bnp[:, 13] = BN_EPS

    in_maps = []
    for c in range(N_CORES):
        invc = np.ascontiguousarray(
            invc_full[c * SEGS_PER_CORE:(c + 1) * SEGS_PER_CORE]
            .reshape(BLOCKS, 128).T)
        m = {
            "emb": emb, "idx16": packed[c][0], "sego": packed[c][1],
            "invc": invc, "tpk": tpk, "wo": Wo_np, "bnp": bnp,
            "iota": iota, "ident": ident,
        }
        for l in range(3):
            m[f"w{l}"] = Ws[l]
        in_maps.append(m)

    global _LAST_IN_MAPS
    _LAST_IN_MAPS = in_maps
    res = run_bass_kernel_spmd(nc, in_maps, core_ids=list(range(N_CORES)))
    z_pk = res.results[0]["z"]
    loss = res.results[0]["loss"].reshape(())
    z = np.ascontiguousarray(z_pk.T.reshape(B)).astype(np.float32)
    return np.float32(loss), z
